# revision 1
# baseline (speedup 1.0000x reference)
"""Cross-attention (GQA) Trainium2 Bass kernel — pipelined v2.

Problem: B=2, Tq=Tkv=2048, D_MODEL=1024, 16 query heads / 4 kv heads,
head_dim=64.  Sharded over 8 NeuronCores as batch(2) x kv-group(4); each
core computes 4 query heads + its single kv head and a partial output
projection (Wo row-split by head group); partials are summed on host.

Dataflow (feature dim on SBUF partitions end-to-end, no big transposes):

  A: qT[e,t] = WqT.T @ xqT,  kvT = WkvT.T @ xcT   (weights stationary)
     v[tk,dv] via PE-transpose of vT tiles; vp=[v|1], vp2=[1|v]
  B: per (blk,e) section, unit t: pb[128,1024] = two K=64 row-group
     matmuls (h_even rows 0-63 -> cols 0:512, h_odd rows 64-127 ->
     cols 512:1024), concurrent in the PE array.
  C: pt = exp(pb/8) one ScalarE instruction per unit (FD=1024).
  D: pd_h[128,512] += vp_t.T @ pt_half; ones-columns give the softmax
     denominators in the complementary 64 partitions.
  E: yT += WoT.T @ (pd*recip(den)), row-split by head pair.

The whole BCD stream is software-pipelined: the PE emission order is
B(t), D(t-1) so matmuls never wait on the ScalarE exp of the same unit;
projection/output-projection matmuls are fed as "fill" work into the
PE slack inside each section.  ScalarE (the 1 elem/cycle/lane exp
bottleneck, ~143us) paces the kernel; the PE stays dense and HAM-warm.
"""

import os
import sys
from collections import deque

import numpy as np

for _p in ("/opt/trn_rl_repo",):
    if _p not in sys.path and os.path.isdir(_p):
        sys.path.insert(0, _p)

import concourse.bass as bass
import concourse.bacc as bacc
import concourse.mybir as mybir
from concourse.tile import TileContext

# ---------------------------------------------------------------- problem dims
B = 2
TQ = 2048
TKV = 2048
D_MODEL = 1024
N_HEADS = 16
N_KV_HEADS = 4
HEAD_DIM = 64
N_CORES = 8
GROUPS = N_KV_HEADS  # kv groups = 4
HEADS_PER_DEV = N_HEADS // GROUPS  # 4
DQ = HEADS_PER_DEV * HEAD_DIM  # 256
DKV = 2 * HEAD_DIM  # 128 (k rows + v rows stacked)
SCALE = 1.0 / float(np.sqrt(HEAD_DIM))

P = 128
FREE = 512  # matmul moving-operand chunk / tq block width
BLK = 512
NBLK = TQ // BLK  # 4 tq blocks
DT = D_MODEL // P  # 8 d-tiles
ET = DQ // P  # 2 e-tiles (query head pairs)
NCH = TQ // FREE  # 4 x chunks of 512
NTK = TKV // P  # 16 tk tiles
MT = D_MODEL // P  # 8 output m-tiles

F32 = mybir.dt.float32
F16 = mybir.dt.float16


def build_bass():
    nc = bacc.Bacc()

    xq = nc.declare_dram_parameter("xqT", [D_MODEL, TQ], F16, isOutput=False)
    xc = nc.declare_dram_parameter("xcT", [D_MODEL, TKV], F16, isOutput=False)
    wq = nc.declare_dram_parameter("wqT", [D_MODEL, DQ], F16, isOutput=False)
    wkv = nc.declare_dram_parameter("wkvT", [D_MODEL, DKV], F16, isOutput=False)
    wo = nc.declare_dram_parameter("woT", [DQ, D_MODEL], F16, isOutput=False)
    cid = nc.declare_dram_parameter("cid", [P, P], F16, isOutput=False)
    yt = nc.declare_dram_parameter("yT", [D_MODEL, TQ], F16, isOutput=True)

    with TileContext(nc) as tc:
        with (
            tc.tile_pool(name="consts", bufs=1) as consts,
            tc.tile_pool(name="xch", bufs=2) as xpool,
            tc.tile_pool(name="pt", bufs=4) as ptpool,
            tc.tile_pool(name="rec", bufs=2) as recpool,
            tc.tile_pool(name="yout", bufs=3) as ypool,
            tc.tile_pool(name="psS", bufs=2, space="PSUM") as psS,
            tc.tile_pool(name="psD", bufs=1, space="PSUM") as psD,
            tc.tile_pool(name="psA", bufs=2, space="PSUM") as psA,
        ):
            # ---------------- constants / persistent tiles
            # DMA priority order: the lead-in critical path is
            # xc0+wkv (kv proj) then xq0+wq (q proj) -> first B matmul.
            qt = consts.tile([P, ET, TQ], F16, tag="qt")  # head pair per e
            kv = consts.tile([P, TKV], F16, tag="kv")  # rows 0-63 kT, 64-127 vT
            k2 = consts.tile([P, TKV], F16, tag="k2")  # rows 64-127 = kT copy
            vp = consts.tile([P, NTK, P], F16, tag="vp")  # [v | ones]
            vp2 = consts.tile([P, NTK, P], F16, tag="vp2")  # [ones | v]
            outs = consts.tile([P, ET, TQ], F16, tag="outs")  # normalized outT

            # input chunk dmas (xpool rotates 2 bufs per tag)
            def dma_xc(c):
                cs = slice(c * FREE, (c + 1) * FREE)
                t = xpool.tile([P, DT, FREE], F16, tag="xc", name=f"xc{c}", bufs=3)
                nc.sync.dma_start(t, xc.rearrange("(i p) t -> p i t", p=P)[:, :, cs])
                return t

            def dma_xq(c):
                cs = slice(c * FREE, (c + 1) * FREE)
                t = xpool.tile([P, DT, FREE], F16, tag="xq", name=f"xq{c}", bufs=3)
                nc.sync.dma_start(t, xq.rearrange("(i p) t -> p i t", p=P)[:, :, cs])
                return t

            ident = consts.tile([P, P], F16, tag="ident")
            nc.sync.dma_start(ident, cid[:])
            xc_t = [None] * NCH
            xq_t = [None] * NCH
            xc_t[0] = dma_xc(0)
            wkv_sb = consts.tile([P, DT, DKV], F16, tag="wkv")
            nc.sync.dma_start(wkv_sb, wkv.rearrange("(i p) e -> p i e", p=P))
            xq_t[0] = dma_xq(0)
            wq_sb = consts.tile([P, DT, DQ], F16, tag="wq")
            nc.sync.dma_start(wq_sb, wq.rearrange("(i p) e -> p i e", p=P))
            wo_sb = consts.tile([P, ET, D_MODEL], F16, tag="wo")

            nc.vector.memset(vp, 1.0)
            nc.vector.memset(vp2, 1.0)

            # Warm-up while input DMAs stream: ~3.5us of dummy matmuls gets
            # the PE HAM clock-gate to 8/8 (2.4GHz) before the projections;
            # a tiny exp pulls the ScalarE ACT table load off the hot path.
            dum = consts.tile([P, 8], F16, tag="dum")
            nc.scalar.activation(
                dum, ident[:, :8], mybir.ActivationFunctionType.Exp, bias=0.0, scale=1.0
            )
            warm = psA.tile([P, P], F32, tag="pa", name="warm")
            for i in range(30):
                nc.tensor.matmul(warm, ident, ident, start=(i == 0), stop=(i == 29))

            # ---------------- fill-work machinery (PE slack consumers)
            fills = deque()
            deferred = deque()  # per-section normalize chains (DVE)

            def pop_fill(n=1):
                for _ in range(n):
                    if not fills:
                        return
                    fills.popleft()()

            # D matmuls for one pipelined unit (two heads, K=128, N=512)
            def emit_d(pd0, pd1, pt, t):
                nc.tensor.matmul(
                    pd0, vp[:, t, :], pt[:, :BLK],
                    start=(t == 0), stop=(t == NTK - 1), skip_group_check=True,
                )
                nc.tensor.matmul(
                    pd1, vp2[:, t, :], pt[:, BLK:],
                    start=(t == 0), stop=(t == NTK - 1), skip_group_check=True,
                )

            # kv projection chunk: 8 K-tiles -> kv[:, cs]; k2 copy; transposes
            def kv_chunk_pieces(c, get_xc):
                cs = slice(c * FREE, (c + 1) * FREE)
                st = {}

                def pk(i0):
                    def p():
                        if i0 == 0:
                            st["pkv"] = psA.tile([P, FREE], F32, tag="pa", name="pkv")
                        for i in range(i0, i0 + 2):
                            nc.tensor.matmul(
                                st["pkv"], wkv_sb[:, i, :], get_xc()[:, i, :],
                                start=(i == 0), stop=(i == DT - 1),
                            )
                        if i0 == DT - 2:
                            nc.vector.tensor_copy(kv[:, cs], st["pkv"])
                            nc.sync.dma_start(k2[HEAD_DIM:, cs], kv[:HEAD_DIM, cs])

                    return p

                def p3():
                    # transpose the 4 v tiles of this chunk, batch-copy to vp/vp2
                    pvb = psA.tile([P, 4 * HEAD_DIM], F16, tag="pa", name="pvb")
                    for k in range(4):
                        ts_ = slice((4 * c + k) * P, (4 * c + k + 1) * P)
                        nc.tensor.transpose(
                            pvb[:, k * HEAD_DIM : (k + 1) * HEAD_DIM],
                            kv[HEAD_DIM:, ts_],
                            ident[HEAD_DIM:, HEAD_DIM:],
                        )
                    src = pvb.rearrange("p (k d) -> p k d", k=4)
                    nc.vector.tensor_copy(vp[:, 4 * c : 4 * c + 4, :HEAD_DIM], src)
                    nc.vector.tensor_copy(vp2[:, 4 * c : 4 * c + 4, HEAD_DIM:], src)

                return [pk(0), pk(2), pk(4), pk(6), p3]

            # q projection chunk (one e): 8 K-tiles -> qt[:, e, cs]
            def q_chunk_pieces(c, e, get_xq):
                cs = slice(c * FREE, (c + 1) * FREE)
                st = {}

                def pq(i0):
                    def p():
                        if i0 == 0:
                            st["pq"] = psA.tile([P, FREE], F32, tag="pa", name="pq")
                        for i in range(i0, i0 + 2):
                            nc.tensor.matmul(
                                st["pq"], wq_sb[:, i, e * P : (e + 1) * P],
                                get_xq()[:, i, :],
                                start=(i == 0), stop=(i == DT - 1),
                            )
                        if i0 == DT - 2:
                            nc.vector.tensor_copy(qt[:, e, cs], st["pq"])

                    return p

                return [pq(0), pq(2), pq(4), pq(6)]

            # output-projection piece for one m-tile of one tq block
            def e_piece(blk, m):
                bs = slice(blk * BLK, (blk + 1) * BLK)
                ms = slice(m * P, (m + 1) * P)

                def p():
                    py = psA.tile([P, FREE], F32, tag="pa", name="py")
                    for ee in range(ET):
                        nc.tensor.matmul(
                            py, wo_sb[:, ee, ms], outs[:, ee, bs],
                            start=(ee == 0), stop=(ee == ET - 1),
                        )
                    yo = ypool.tile([P, FREE], F16, tag="yo", name="yo")
                    nc.vector.tensor_copy(yo, py)
                    nc.sync.dma_start(yt[ms, bs], yo)

                return p

            # ---------------- lead-in: minimum inline work before section 0:
            # kv c0 projection + q c0 (e=0 only); everything else is fills.
            kc0 = kv_chunk_pieces(0, lambda: xc_t[0])
            for piece in kc0[:4]:
                piece()
            for piece in q_chunk_pieces(0, 0, lambda: xq_t[0]):
                piece()

            # Fill order is a DEADLINE order: section-0 units consume 2
            # pieces/unit and a piece's writes are only visible to LATER-
            # emitted readers (Tile deps follow emission order).  kv chunk c
            # must be fully emitted before B(t=4c); vp transposes for chunk c
            # before D(4c), which lags B by 2 units.
            xc_t[1] = dma_xc(1)
            kc1 = kv_chunk_pieces(1, lambda: xc_t[1])
            for piece in kc1[:4]:
                piece()
            fills.append(lambda: xq_t.__setitem__(1, dma_xq(1)))
            fills.append(kc0[4])  # v transposes for chunk 0 (D(0) is unit 2)
            fills.append(kc1[4])
            # xc2/xc3 dmas go AFTER kv c1's k2 broadcast so they don't block
            # it in the single sync-DMA queue
            fills.append(lambda: xc_t.__setitem__(2, dma_xc(2)))
            fills.append(lambda: xc_t.__setitem__(3, dma_xc(3)))
            fills.extend(kv_chunk_pieces(2, lambda: xc_t[2]))
            fills.extend(kv_chunk_pieces(3, lambda: xc_t[3]))
            fills.extend(q_chunk_pieces(0, 1, lambda: xq_t[0]))
            fills.append(
                lambda: nc.sync.dma_start(wo_sb, wo.rearrange("(i p) m -> p i m", p=P))
            )
            for e in range(ET):
                fills.extend(q_chunk_pieces(1, e, lambda: xq_t[1]))

            # ---------------- BCD sections
            for sec, (blk, e) in enumerate(
                (blk, e) for blk in range(NBLK) for e in range(ET)
            ):
                bs = slice(blk * BLK, (blk + 1) * BLK)
                pd0 = psD.tile([P, BLK], F32, tag="pd0", name="pd0")
                pd1 = psD.tile([P, BLK], F32, tag="pd1", name="pd1")
                pending = deque()  # D lags B by 2 units so exp sems are settled
                for t in range(NTK):
                    ts_ = slice(t * P, (t + 1) * P)
                    pb = psS.tile([P, 2 * BLK], F32, tag="pb", name="pb")
                    # B: two K=64 row-group matmuls, concurrent in the array
                    nc.tensor.matmul(pb[:, :BLK], kv[:HEAD_DIM, ts_], qt[:HEAD_DIM, e, bs])
                    nc.tensor.matmul(pb[:, BLK:], k2[HEAD_DIM:, ts_], qt[HEAD_DIM:, e, bs])
                    if len(pending) >= 2:
                        emit_d(*pending.popleft())
                    pt = ptpool.tile([P, 2 * BLK], F16, tag="pt", name="pt")
                    nc.scalar.activation(
                        pt, pb, mybir.ActivationFunctionType.Exp, bias=0.0, scale=SCALE
                    )
                    if t == 8 and deferred:
                        deferred.popleft()()
                    if sec == 0:
                        pop_fill(2)
                    elif t >= 2:
                        pop_fill(1)
                    pending.append((pd0, pd1, pt, t))
                while pending:
                    emit_d(*pending.popleft())

                # normalize: spill pd fast (frees PSUM for the next section).
                # The slow recip+mul chain is DEFERRED into the middle of the
                # next section's DVE stream: Tile's waits are thresholds on a
                # single DVE completion counter, so any fill matmul emitted
                # after a 3.3us reciprocal transitively waits for it.  Keeping
                # the chain out of the stream until the next section's fills
                # have been emitted removes those ~6us/section stalls.
                # The last section skips the spill and runs inline (tail).
                last = sec == NBLK * ET - 1
                if last:
                    raw0, raw1 = pd0, pd1
                else:
                    raw0 = recpool.tile([P, BLK], F32, tag="raw0", name="raw0")
                    raw1 = recpool.tile([P, BLK], F32, tag="raw1", name="raw1")
                    nc.vector.tensor_copy(raw0, pd0)
                    nc.vector.tensor_copy(raw1, pd1)

                def norm_chain(e=e, bs=bs, raw0=raw0, raw1=raw1):
                    rec0 = recpool.tile([P, BLK], F32, tag="rec0", name="rec0")
                    rec1 = recpool.tile([P, BLK], F32, tag="rec1", name="rec1")
                    nc.vector.reciprocal(rec0[HEAD_DIM:, :], raw0[HEAD_DIM:, :])
                    nc.sync.dma_start(rec0[:HEAD_DIM, :], rec0[HEAD_DIM:, :])
                    nc.vector.reciprocal(rec1[:HEAD_DIM, :], raw1[:HEAD_DIM, :])
                    nc.vector.tensor_mul(
                        outs[:HEAD_DIM, e, bs], raw0[:HEAD_DIM, :], rec0[:HEAD_DIM, :]
                    )
                    nc.sync.dma_start(rec1[HEAD_DIM:, :], rec1[:HEAD_DIM, :])
                    nc.vector.tensor_mul(
                        outs[HEAD_DIM:, e, bs], raw1[HEAD_DIM:, :], rec1[HEAD_DIM:, :]
                    )

                if last:
                    norm_chain()
                else:
                    deferred.append(norm_chain)

                # queue follow-on work.  x dmas issue a full section before
                # their consuming projection pieces; E pieces go LAST so they
                # pop only after the producing normalize has finished (an
                # early E piece stalls the in-order PE on outs and lets the
                # HAM clock-gate go cold).
                if sec == 0:
                    fills.append(lambda: xq_t.__setitem__(2, dma_xq(2)))
                if sec == 1:
                    for ee in range(ET):
                        fills.extend(q_chunk_pieces(2, ee, lambda: xq_t[2]))
                if sec == 2:
                    fills.append(lambda: xq_t.__setitem__(3, dma_xq(3)))
                if sec == 3:
                    for ee in range(ET):
                        fills.extend(q_chunk_pieces(3, ee, lambda: xq_t[3]))
                # E(b) pieces are deferred a FULL section past b's last
                # normalize so they never stall the in-order PE on outs
                if sec in (2, 4, 6):
                    for m in range(MT):
                        fills.append(e_piece(sec // 2 - 1, m))
                if sec == NBLK * ET - 1:
                    for m in range(MT):
                        fills.append(e_piece(blk, m))

            # tail: keep the PE clocked while the final normalize chain runs
            # on the DVE, then drain the last block's output projection
            while deferred:
                deferred.popleft()()
            wtail = psA.tile([P, P], F32, tag="pa", name="wtail")
            for i in range(30):
                nc.tensor.matmul(wtail, ident, ident, start=(i == 0), stop=(i == 29))
            while fills:
                pop_fill()

    nc.finalize()
    return nc


_NC_CACHE = None


def _get_nc():
    global _NC_CACHE
    if _NC_CACHE is None:
        _NC_CACHE = build_bass()
    return _NC_CACHE


def _cid():
    return np.eye(P, dtype=np.float16)


def shard_inputs(query, context, Wq, Wk, Wv, Wo):
    """host-side sharding: 8 cores = batch(2) x kv-group(4)"""
    in_maps = []
    xqT = [np.ascontiguousarray(query[b].T).astype(np.float16) for b in range(B)]
    xcT = [np.ascontiguousarray(context[b].T).astype(np.float16) for b in range(B)]
    for core in range(N_CORES):
        b, g = divmod(core, GROUPS)
        wqT = np.ascontiguousarray(Wq[g * DQ : (g + 1) * DQ, :].T).astype(np.float16)
        wkvT = np.ascontiguousarray(
            np.concatenate(
                [
                    Wk[g * HEAD_DIM : (g + 1) * HEAD_DIM, :],
                    Wv[g * HEAD_DIM : (g + 1) * HEAD_DIM, :],
                ],
                axis=0,
            ).T
        ).astype(np.float16)
        woT = np.ascontiguousarray(Wo[:, g * DQ : (g + 1) * DQ].T).astype(np.float16)
        in_maps.append(
            {
                "xqT": xqT[b],
                "xcT": xcT[b],
                "wqT": wqT,
                "wkvT": wkvT,
                "woT": woT,
                "cid": _cid(),
            }
        )
    return in_maps


def kernel(query, context, Wq, Wk, Wv, Wo, _want_profile=False):
    from concourse.bass_utils import run_bass_kernel_spmd

    nc = _get_nc()
    in_maps = shard_inputs(query, context, Wq, Wk, Wv, Wo)
    res = run_bass_kernel_spmd(
        nc, in_maps, core_ids=list(range(N_CORES)), trace=_want_profile
    )
    out = np.zeros((B, TQ, D_MODEL), dtype=np.float32)
    for core in range(N_CORES):
        b = core // GROUPS
        out[b] += res.results[core]["yT"].T.astype(np.float32)
    if _want_profile:
        return out, res
    return out



# revision 6
# speedup vs baseline: 1.1043x; 1.1043x over previous
"""Cross-attention (GQA) Trainium2 Bass kernel — pipelined v3.

Problem: B=2, Tq=Tkv=2048, D_MODEL=1024, 16 query heads / 4 kv heads,
head_dim=64.  Sharded over 8 NeuronCores as batch(2) x kv-group(4); each
core computes 4 query heads + its single kv head and a partial output
projection (Wo row-split by head group); partials are summed on host.

Dataflow (feature dim on SBUF partitions end-to-end, no big transposes):

  A: qT[e,t] = WqT.T @ xqT,  kvT = WkvT.T @ xcT   (weights stationary)
     v[tk,dv] via PE-transpose of vT tiles; vp=[v|1], vp2=[1|v]
  B: per (sec=blk,e) unit t: pb[128,1024] = two K=64 row-group matmuls
     (h_even rows 0-63 -> cols 0:512, h_odd rows 64-127 -> cols
     512:1024), concurrent in the PE array.
  C: pt = exp(pb/8) one ScalarE instruction per unit (FD=1024).
  D: pd_h[128,512] += vp_t.T @ pt_half; ones-columns give the softmax
     denominators in the complementary 64 partitions.
  E: yT += WoT.T @ (pd*recip(den)), row-split by head pair.

v3 over v2:
  - ONE continuous 128-unit software pipeline (B(u), D(u-2), exp(u))
    across all 8 sections: no PE drain / exp restart bubble at section
    boundaries.
  - reciprocal_approx_fast (custom DVE op, ~5x faster than RECIPROCAL)
    in the softmax-denominator normalize chains.
  - Input DMAs split across BOTH hardware DGE queues: xq/wq/wo/ident
    issue from the (idle during lead-in) scalar/Act queue, xc/wkv from
    the sync queue, all unconditionally at kernel start so no waiting
    DMA ever sits in front of an input load.
  - Norm chains and output-projection (E) pieces pop from a dedicated
    "late" queue only at units t in [5..14] of the FOLLOWING section,
    when their DVE-side producers have long retired - they never stall
    the in-order PE.
  - y stores staged per tq-block in SBUF and written with 2 half-block
    DMAs (8 DMA issues/block -> 2), keeping the sync queue uncongested
    so the rec-broadcast DMAs on the norm critical path fire promptly.

ScalarE (the 1 elem/cycle/lane exp bottleneck, ~143us) paces the
kernel; the PE stays dense and HAM-warm.
"""

import os
import sys
from collections import deque

import numpy as np

for _p in ("/opt/trn_rl_repo",):
    if _p not in sys.path and os.path.isdir(_p):
        sys.path.insert(0, _p)

import concourse.bass as bass
import concourse.bacc as bacc
import concourse.mybir as mybir
from concourse.tile import TileContext

# ---------------------------------------------------------------- problem dims
B = 2
TQ = 2048
TKV = 2048
D_MODEL = 1024
N_HEADS = 16
N_KV_HEADS = 4
HEAD_DIM = 64
N_CORES = 8
GROUPS = N_KV_HEADS  # kv groups = 4
HEADS_PER_DEV = N_HEADS // GROUPS  # 4
DQ = HEADS_PER_DEV * HEAD_DIM  # 256
DKV = 2 * HEAD_DIM  # 128 (k rows + v rows stacked)
SCALE = 1.0 / float(np.sqrt(HEAD_DIM))

P = 128
FREE = 512  # matmul moving-operand chunk / tq block width
BLK = 512
NBLK = TQ // BLK  # 4 tq blocks
DT = D_MODEL // P  # 8 d-tiles
ET = DQ // P  # 2 e-tiles (query head pairs)
NCH = TQ // FREE  # 4 x chunks of 512
NTK = TKV // P  # 16 tk tiles
MT = D_MODEL // P  # 8 output m-tiles
NSEC = NBLK * ET  # 8 sections
NU = NSEC * NTK  # 128 pipelined units

F32 = mybir.dt.float32
F16 = mybir.dt.float16


def build_bass():
    nc = bacc.Bacc()

    xq = nc.declare_dram_parameter("xqT", [D_MODEL, TQ], F16, isOutput=False)
    xc = nc.declare_dram_parameter("xcT", [D_MODEL, TKV], F16, isOutput=False)
    wq = nc.declare_dram_parameter("wqT", [D_MODEL, DQ], F16, isOutput=False)
    wkv = nc.declare_dram_parameter("wkvT", [D_MODEL, DKV], F16, isOutput=False)
    wo = nc.declare_dram_parameter("woT", [DQ, D_MODEL], F16, isOutput=False)
    cid = nc.declare_dram_parameter("cid", [P, P], F16, isOutput=False)
    yt = nc.declare_dram_parameter("yT", [D_MODEL, TQ], F16, isOutput=True)
    ytr = yt.rearrange("(i p) t -> p i t", p=P)

    with TileContext(nc) as tc:
        with (
            tc.tile_pool(name="consts", bufs=1) as consts,
            tc.tile_pool(name="pt", bufs=4) as ptpool,
            tc.tile_pool(name="rec", bufs=2) as recpool,
            tc.tile_pool(name="yout", bufs=2) as ypool,
            tc.tile_pool(name="psS", bufs=2, space="PSUM") as psS,
            tc.tile_pool(name="psD", bufs=1, space="PSUM") as psD,
            tc.tile_pool(name="psA", bufs=2, space="PSUM") as psA,
        ):
            # ---------------- persistent tiles
            qt = consts.tile([P, ET, TQ], F16, tag="qt")  # head pair per e
            kv = consts.tile([P, TKV], F16, tag="kv")  # rows 0-63 kT, 64-127 vT
            k2 = consts.tile([P, TKV], F16, tag="k2")  # rows 64-127 = kT copy
            vp = consts.tile([P, NTK, P], F16, tag="vp")  # [v | ones]
            vp2 = consts.tile([P, NTK, P], F16, tag="vp2")  # [ones | v]
            outs = consts.tile([P, ET, TQ], F16, tag="outs")  # normalized outT
            ident = consts.tile([P, P], F16, tag="ident")
            wkv_sb = consts.tile([P, DT, DKV], F16, tag="wkv")
            wq_sb = consts.tile([P, DT, DQ], F16, tag="wq")
            wo_sb = consts.tile([P, ET, D_MODEL], F16, tag="wo")
            wrm = consts.tile([P, P], F16, tag="wrm")
            xq_t = [
                consts.tile([P, DT, FREE], F16, tag=f"xq{c}", name=f"xq{c}")
                for c in range(NCH)
            ]
            xc_t = [
                consts.tile([P, DT, FREE], F16, tag=f"xc{c}", name=f"xc{c}")
                for c in range(NCH)
            ]

            # ---------------- input DMA burst: everything issues up front,
            # split across the two hardware DGE queues.  Neither queue has a
            # waiting DMA in front of an input load.  Most-urgent first.
            xqr = xq.rearrange("(i p) t -> p i t", p=P)
            xcr = xc.rearrange("(i p) t -> p i t", p=P)
            # sync queue: kv path (xc chunks feed the B-matmul deadline chain)
            nc.sync.dma_start(xc_t[0], xcr[:, :, 0:FREE])
            nc.sync.dma_start(wkv_sb, wkv.rearrange("(i p) e -> p i e", p=P))
            for c in range(1, NCH):
                nc.sync.dma_start(xc_t[c], xcr[:, :, c * FREE : (c + 1) * FREE])
            # scalar/Act queue: q path + consts (engine idle until first exp)
            nc.scalar.dma_start(xq_t[0], xqr[:, :, 0:FREE])
            nc.scalar.dma_start(wq_sb, wq.rearrange("(i p) e -> p i e", p=P))
            nc.scalar.dma_start(ident, cid[:])
            for c in range(1, NCH):
                nc.scalar.dma_start(xq_t[c], xqr[:, :, c * FREE : (c + 1) * FREE])
            nc.scalar.dma_start(wo_sb, wo.rearrange("(i p) m -> p i m", p=P))

            # vector-side const init + HAM warm-up (no DMA dependencies:
            # wrm memset feeds dummy matmuls that spin the PE clock up while
            # the inputs stream; a tiny exp pulls the ACT table load early).
            nc.vector.memset(wrm, 0.125)
            nc.vector.memset(vp, 1.0)
            nc.vector.memset(vp2, 1.0)
            dum = consts.tile([P, 8], F16, tag="dum")
            nc.scalar.activation(
                dum, wrm[:, :8], mybir.ActivationFunctionType.Exp, bias=0.0, scale=1.0
            )
            warm = psA.tile([P, P], F32, tag="pa", name="warm")
            for i in range(16):
                nc.tensor.matmul(warm, wrm, wrm, start=(i == 0), stop=(i == 15))

            # ---------------- fill-work machinery (PE slack consumers)
            fills = deque()  # anytime work: projections, transposes, dma issues
            late = deque()  # dep-settled-late work: norm chains, E pieces

            def pop_fill(n=1):
                for _ in range(n):
                    if not fills:
                        return
                    fills.popleft()()

            # D matmuls for one pipelined unit (two heads, K=128, N=512)
            def emit_d(pd0, pd1, pt, t):
                nc.tensor.matmul(
                    pd0, vp[:, t, :], pt[:, :BLK],
                    start=(t == 0), stop=(t == NTK - 1), skip_group_check=True,
                )
                nc.tensor.matmul(
                    pd1, vp2[:, t, :], pt[:, BLK:],
                    start=(t == 0), stop=(t == NTK - 1), skip_group_check=True,
                )

            # kv projection chunk: 8 K-tiles -> kv[:, cs]; k2 bcast; transposes
            def kv_chunk_pieces(c):
                cs = slice(c * FREE, (c + 1) * FREE)
                st = {}

                def pk(i0):
                    def p():
                        if i0 == 0:
                            st["pkv"] = psA.tile([P, FREE], F32, tag="pa", name="pkv")
                        for i in range(i0, i0 + 2):
                            nc.tensor.matmul(
                                st["pkv"], wkv_sb[:, i, :], xc_t[c][:, i, :],
                                start=(i == 0), stop=(i == DT - 1),
                            )
                        if i0 == DT - 2:
                            nc.vector.tensor_copy(kv[:, cs], st["pkv"])
                            nc.sync.dma_start(k2[HEAD_DIM:, cs], kv[:HEAD_DIM, cs])

                    return p

                def p3():
                    # transpose the 4 v tiles of this chunk, batch-copy to vp/vp2
                    pvb = psA.tile([P, 4 * HEAD_DIM], F16, tag="pa", name="pvb")
                    for k in range(4):
                        ts_ = slice((4 * c + k) * P, (4 * c + k + 1) * P)
                        nc.tensor.transpose(
                            pvb[:, k * HEAD_DIM : (k + 1) * HEAD_DIM],
                            kv[HEAD_DIM:, ts_],
                            ident[HEAD_DIM:, HEAD_DIM:],
                        )
                    src = pvb.rearrange("p (k d) -> p k d", k=4)
                    nc.vector.tensor_copy(vp[:, 4 * c : 4 * c + 4, :HEAD_DIM], src)
                    nc.vector.tensor_copy(vp2[:, 4 * c : 4 * c + 4, HEAD_DIM:], src)

                return [pk(0), pk(2), pk(4), pk(6), p3]

            # q projection chunk (one e): 8 K-tiles -> qt[:, e, cs]
            def q_chunk_pieces(c, e):
                cs = slice(c * FREE, (c + 1) * FREE)
                st = {}

                def pq(i0):
                    def p():
                        if i0 == 0:
                            st["pq"] = psA.tile([P, FREE], F32, tag="pa", name="pq")
                        for i in range(i0, i0 + 2):
                            nc.tensor.matmul(
                                st["pq"], wq_sb[:, i, e * P : (e + 1) * P],
                                xq_t[c][:, i, :],
                                start=(i == 0), stop=(i == DT - 1),
                            )
                        if i0 == DT - 2:
                            nc.vector.tensor_copy(qt[:, e, cs], st["pq"])

                    return p

                return [pq(0), pq(2), pq(4), pq(6)]

            # output projection for one tq block: 8 m-tile pieces staged into
            # ySB, then 2 half-block DMA issues.
            def e_pieces(blk):
                bs = slice(blk * BLK, (blk + 1) * BLK)
                ysb = ypool.tile([P, MT, FREE], F16, tag="ysb", name=f"ysb{blk}")

                def mk(m):
                    def p():
                        py = psA.tile([P, FREE], F32, tag="pa", name="py")
                        for ee in range(ET):
                            nc.tensor.matmul(
                                py, wo_sb[:, ee, m * P : (m + 1) * P], outs[:, ee, bs],
                                start=(ee == 0), stop=(ee == ET - 1),
                            )
                        nc.vector.tensor_copy(ysb[:, m, :], py)

                    return p

                def store(h):
                    def p():
                        ms = slice(h * (MT // 2), (h + 1) * (MT // 2))
                        nc.sync.dma_start(ytr[:, ms, bs], ysb[:, ms, :])

                    return p

                return [mk(m) for m in range(MT)] + [store(0), store(1)]

            # spill pd0/pd1 for one section into aligned full-partition
            # tiles: rawN = [AV_even | AV_odd], rawD = [den_odd | den_even]
            # (reciprocal_approx_fast silently corrupts partition-offset
            # operands, so the custom op must see full offset-0 tiles).
            def spill(dp0, dp1):
                rawN = recpool.tile([P, BLK], F32, tag="rawN", name="rawN")
                rawD = recpool.tile([P, BLK], F32, tag="rawD", name="rawD")
                nc.vector.tensor_copy(rawN[:HEAD_DIM, :], dp0[:HEAD_DIM, :])
                nc.vector.tensor_copy(rawN[HEAD_DIM:, :], dp1[HEAD_DIM:, :])
                nc.vector.tensor_copy(rawD[:HEAD_DIM, :], dp1[:HEAD_DIM, :])
                nc.vector.tensor_copy(rawD[HEAD_DIM:, :], dp0[HEAD_DIM:, :])
                return rawN, rawD

            # normalize chain for one section (deferred into the late window
            # of the following section): one full-partition fast recip, two
            # half-swap broadcast DMAs, one full-width multiply.
            def norm_chain(sec, rawN, rawD):
                blk, e = divmod(sec, ET)
                bs = slice(blk * BLK, (blk + 1) * BLK)

                def p():
                    recD = recpool.tile([P, BLK], F32, tag="recD", name="recD")
                    recS = recpool.tile([P, BLK], F32, tag="recS", name="recS")
                    nc.vector.reciprocal_approx_fast(recD, rawD)
                    nc.sync.dma_start(recS[:HEAD_DIM, :], recD[HEAD_DIM:, :])
                    nc.sync.dma_start(recS[HEAD_DIM:, :], recD[:HEAD_DIM, :])
                    nc.vector.tensor_mul(outs[:, e, bs], rawN, recS)

                return p

            # ---------------- lead-in PE work: kv chunk 0 + q chunk 0 (e=0)
            # inline; everything else is fills with deadline-ordered layout.
            kc0 = kv_chunk_pieces(0)
            for piece in kc0[:4]:
                piece()
            for piece in q_chunk_pieces(0, 0):
                piece()

            # Fill deadline order for section 0 (2 pops/unit):
            #   kc0.p3 (v transposes, before D(0) at u=2) -> t<=1
            #   kc1 pk pieces before B(4) -> t<=3 ; kc1.p3 before D(4) -> t<=5
            #   kc2 before B(8); kc3 before B(12); q(0,1) before section 1.
            kcs = [None, kv_chunk_pieces(1), kv_chunk_pieces(2), kv_chunk_pieces(3)]
            fills.append(kc0[4])
            fills.extend(kcs[1][:4])
            fills.append(kcs[1][4])
            fills.extend(kcs[2][:5])
            fills.extend(kcs[3][:5])
            fills.extend(q_chunk_pieces(0, 1))
            for e in range(ET):
                fills.extend(q_chunk_pieces(1, e))
            for e in range(ET):
                fills.extend(q_chunk_pieces(2, e))
            for e in range(ET):
                fills.extend(q_chunk_pieces(3, e))

            # ---------------- the continuous BCD pipeline over 128 units
            units = [(sec, t) for sec in range(NSEC) for t in range(NTK)]
            pending = deque()  # (pd0, pd1, pt, t, sec)
            pd_cur = None
            for u, (sec, t) in enumerate(units):
                blk, e = divmod(sec, ET)
                bs = slice(blk * BLK, (blk + 1) * BLK)
                if t == 0:
                    pd_cur = (
                        psD.tile([P, BLK], F32, tag="pd0", name="pd0"),
                        psD.tile([P, BLK], F32, tag="pd1", name="pd1"),
                    )
                pb = psS.tile([P, 2 * BLK], F32, tag="pb", name="pb")
                # B: two K=64 row-group matmuls, concurrent in the array
                nc.tensor.matmul(pb[:, :BLK], kv[:HEAD_DIM, t * P : (t + 1) * P],
                                 qt[:HEAD_DIM, e, bs])
                nc.tensor.matmul(pb[:, BLK:], k2[HEAD_DIM:, t * P : (t + 1) * P],
                                 qt[HEAD_DIM:, e, bs])
                if len(pending) >= 2:
                    dp0, dp1, dpt, dt_, dsec = pending.popleft()
                    emit_d(dp0, dp1, dpt, dt_)
                    if dt_ == NTK - 1 and dsec < NSEC - 1:
                        # section dsec fully accumulated: spill pd -> raw
                        # (frees the PSUM banks for this section's own Ds),
                        # queue the normalize + block-complete E work.
                        rawN, rawD = spill(dp0, dp1)
                        late.append(norm_chain(dsec, rawN, rawD))
                        if dsec % ET == ET - 1 and dsec >= 1:
                            late.extend(e_pieces(dsec // ET))
                pt = ptpool.tile([P, 2 * BLK], F16, tag="pt", name="pt")
                nc.scalar.activation(
                    pt, pb, mybir.ActivationFunctionType.Exp, bias=0.0, scale=SCALE
                )
                pending.append((pd_cur[0], pd_cur[1], pt, t, sec))
                if sec == 0:
                    pop_fill(2)
                elif 5 <= t <= 14 and late:
                    late.popleft()()
                elif t >= 2:
                    pop_fill(1)

            # ---------------- tail: drain last two Ds, normalize the final
            # section straight out of PSUM (fast recip), output-project the
            # last block, store.
            final_pd = None
            while pending:
                dp0, dp1, dpt, dt_, dsec = pending.popleft()
                emit_d(dp0, dp1, dpt, dt_)
                final_pd = (dp0, dp1)
            while late:
                late.popleft()()
            rawN, rawD = spill(final_pd[0], final_pd[1])
            norm_chain(NSEC - 1, rawN, rawD)()
            for piece in e_pieces(NBLK - 1):
                piece()
            while fills:
                pop_fill()

    nc.finalize()
    return nc


_NC_CACHE = None


def _get_nc():
    global _NC_CACHE
    if _NC_CACHE is None:
        _NC_CACHE = build_bass()
    return _NC_CACHE


def _cid():
    return np.eye(P, dtype=np.float16)


def shard_inputs(query, context, Wq, Wk, Wv, Wo):
    """host-side sharding: 8 cores = batch(2) x kv-group(4)"""
    in_maps = []
    xqT = [np.ascontiguousarray(query[b].T).astype(np.float16) for b in range(B)]
    xcT = [np.ascontiguousarray(context[b].T).astype(np.float16) for b in range(B)]
    for core in range(N_CORES):
        b, g = divmod(core, GROUPS)
        wqT = np.ascontiguousarray(Wq[g * DQ : (g + 1) * DQ, :].T).astype(np.float16)
        wkvT = np.ascontiguousarray(
            np.concatenate(
                [
                    Wk[g * HEAD_DIM : (g + 1) * HEAD_DIM, :],
                    Wv[g * HEAD_DIM : (g + 1) * HEAD_DIM, :],
                ],
                axis=0,
            ).T
        ).astype(np.float16)
        woT = np.ascontiguousarray(Wo[:, g * DQ : (g + 1) * DQ].T).astype(np.float16)
        in_maps.append(
            {
                "xqT": xqT[b],
                "xcT": xcT[b],
                "wqT": wqT,
                "wkvT": wkvT,
                "woT": woT,
                "cid": _cid(),
            }
        )
    return in_maps


def kernel(query, context, Wq, Wk, Wv, Wo, _want_profile=False):
    from concourse.bass_utils import run_bass_kernel_spmd

    nc = _get_nc()
    in_maps = shard_inputs(query, context, Wq, Wk, Wv, Wo)
    res = run_bass_kernel_spmd(
        nc, in_maps, core_ids=list(range(N_CORES)), trace=_want_profile
    )
    out = np.zeros((B, TQ, D_MODEL), dtype=np.float32)
    for core in range(N_CORES):
        b = core // GROUPS
        out[b] += res.results[core]["yT"].T.astype(np.float32)
    if _want_profile:
        return out, res
    return out


# revision 13
# speedup vs baseline: 1.1536x; 1.0446x over previous
"""Cross-attention (GQA) Trainium2 Bass kernel — pipelined v3.

Problem: B=2, Tq=Tkv=2048, D_MODEL=1024, 16 query heads / 4 kv heads,
head_dim=64.  Sharded over 8 NeuronCores as batch(2) x kv-group(4); each
core computes 4 query heads + its single kv head and a partial output
projection (Wo row-split by head group); partials are summed on host.

Dataflow (feature dim on SBUF partitions end-to-end, no big transposes):

  A: qT[e,t] = WqT.T @ xqT,  kvT = WkvT.T @ xcT   (weights stationary)
     v[tk,dv] via PE-transpose of vT tiles; vp=[v|1], vp2=[1|v]
  B: per (sec=blk,e) unit t: pb[128,1024] = two K=64 row-group matmuls
     (h_even rows 0-63 -> cols 0:512, h_odd rows 64-127 -> cols
     512:1024), concurrent in the PE array.
  C: pt = exp(pb/8) one ScalarE instruction per unit (FD=1024).
  D: pd_h[128,512] += vp_t.T @ pt_half; ones-columns give the softmax
     denominators in the complementary 64 partitions.
  E: yT += WoT.T @ (pd*recip(den)), row-split by head pair.

v3 over v2:
  - ONE continuous 128-unit software pipeline (B(u), D(u-2), exp(u))
    across all 8 sections: no PE drain / exp restart bubble at section
    boundaries.
  - reciprocal_approx_fast (custom DVE op, ~5x faster than RECIPROCAL)
    in the softmax-denominator normalize chains.
  - Input DMAs split across BOTH hardware DGE queues: xq/wq/wo/ident
    issue from the (idle during lead-in) scalar/Act queue, xc/wkv from
    the sync queue, all unconditionally at kernel start so no waiting
    DMA ever sits in front of an input load.
  - Norm chains and output-projection (E) pieces pop from a dedicated
    "late" queue only at units t in [5..14] of the FOLLOWING section,
    when their DVE-side producers have long retired - they never stall
    the in-order PE.
  - y stores staged per tq-block in SBUF and written with 2 half-block
    DMAs (8 DMA issues/block -> 2), keeping the sync queue uncongested
    so the rec-broadcast DMAs on the norm critical path fire promptly.

ScalarE (the 1 elem/cycle/lane exp bottleneck, ~143us) paces the
kernel; the PE stays dense and HAM-warm.
"""

import os
import sys
from collections import deque

import numpy as np

for _p in ("/opt/trn_rl_repo",):
    if _p not in sys.path and os.path.isdir(_p):
        sys.path.insert(0, _p)

import concourse.bass as bass
import concourse.bacc as bacc
import concourse.mybir as mybir
from concourse.tile import TileContext

# ---------------------------------------------------------------- problem dims
B = 2
TQ = 2048
TKV = 2048
D_MODEL = 1024
N_HEADS = 16
N_KV_HEADS = 4
HEAD_DIM = 64
N_CORES = 8
GROUPS = N_KV_HEADS  # kv groups = 4
HEADS_PER_DEV = N_HEADS // GROUPS  # 4
DQ = HEADS_PER_DEV * HEAD_DIM  # 256
DKV = 2 * HEAD_DIM  # 128 (k rows + v rows stacked)
SCALE = 1.0 / float(np.sqrt(HEAD_DIM))

P = 128
FREE = 512  # matmul moving-operand chunk / tq block width
BLK = 512
NBLK = TQ // BLK  # 4 tq blocks
DT = D_MODEL // P  # 8 d-tiles
ET = DQ // P  # 2 e-tiles (query head pairs)
NCH = TQ // FREE  # 4 x chunks of 512
NTK = TKV // P  # 16 tk tiles
MT = D_MODEL // P  # 8 output m-tiles
NSEC = NBLK * ET  # 8 sections
NU = NSEC * NTK  # 128 pipelined units

F32 = mybir.dt.float32
F16 = mybir.dt.float16


def build_bass():
    nc = bacc.Bacc()

    # all inputs/outputs are host-pre-arranged to be contiguous per SBUF
    # partition: each load/store is ~128 large descriptors, not 1024 small
    # ones (DGE issue cost and HBM efficiency both scale with that).
    xq = nc.declare_dram_parameter("xqh", [P, NCH, DT, FREE], F16, isOutput=False)
    xc = nc.declare_dram_parameter("xch", [P, NCH, DT, FREE], F16, isOutput=False)
    wq = nc.declare_dram_parameter("wqh", [P, DT, DQ], F16, isOutput=False)
    wkv = nc.declare_dram_parameter("wkvh", [P, DT, DKV], F16, isOutput=False)
    wo = nc.declare_dram_parameter("woh", [P, ET, D_MODEL], F16, isOutput=False)
    cid = nc.declare_dram_parameter("cid", [P, P], F16, isOutput=False)
    yt = nc.declare_dram_parameter("yh", [P, NBLK, MT, FREE], F16, isOutput=True)

    with TileContext(nc) as tc:
        with (
            tc.tile_pool(name="consts", bufs=1) as consts,
            tc.tile_pool(name="pt", bufs=4) as ptpool,
            tc.tile_pool(name="rec", bufs=2) as recpool,
            tc.tile_pool(name="yout", bufs=2) as ypool,
            tc.tile_pool(name="psS", bufs=2, space="PSUM") as psS,
            tc.tile_pool(name="psD", bufs=1, space="PSUM") as psD,
            tc.tile_pool(name="psA", bufs=2, space="PSUM") as psA,
        ):
            # ---------------- persistent tiles
            qt = consts.tile([P, ET, TQ], F16, tag="qt")  # head pair per e
            kv = consts.tile([P, TKV], F16, tag="kv")  # rows 0-63 kT, 64-127 vT
            k2 = consts.tile([P, TKV], F16, tag="k2")  # rows 64-127 = kT copy
            vp = consts.tile([P, NTK, P], F16, tag="vp")  # [v | ones]
            vp2 = consts.tile([P, NTK, P], F16, tag="vp2")  # [ones | v]
            outs = consts.tile([P, ET, TQ], F16, tag="outs")  # normalized outT
            ident = consts.tile([P, P], F16, tag="ident")
            wkv_sb = consts.tile([P, DT, DKV], F16, tag="wkv")
            wq_sb = consts.tile([P, DT, DQ], F16, tag="wq")
            wo_sb = consts.tile([P, ET, D_MODEL], F16, tag="wo")
            wrm = consts.tile([P, P], F16, tag="wrm")
            xq_t = [
                consts.tile([P, DT, FREE], F16, tag=f"xq{c}", name=f"xq{c}")
                for c in range(NCH)
            ]
            xc_t = [
                consts.tile([P, DT, FREE], F16, tag=f"xc{c}", name=f"xc{c}")
                for c in range(NCH)
            ]

            # ---------------- input DMA burst: everything issues up front,
            # split across the two hardware DGE queues.  Neither queue has a
            # waiting DMA in front of an input load.  Most-urgent first.
            # sync queue: kv path (xc chunks feed the B-matmul deadline
            # chain), then the non-urgent q-path chunks.
            nc.sync.dma_start(wkv_sb, wkv[:])
            for c in range(NCH):
                nc.sync.dma_start(xc_t[c], xc[:, c, :, :])
            for c in range(1, NCH):
                nc.sync.dma_start(xq_t[c], xq[:, c, :, :])
            nc.sync.dma_start(wo_sb, wo[:])
            # scalar/Act queue (idle until the first exp): only what the
            # first B matmul needs, so the issue cost never delays exp(0).
            nc.scalar.dma_start(xq_t[0], xq[:, 0, :, :])
            nc.scalar.dma_start(wq_sb, wq[:])
            nc.scalar.dma_start(ident, cid[:])

            # vector-side const init + HAM warm-up (no DMA dependencies:
            # wrm memset feeds dummy matmuls that spin the PE clock up while
            # the inputs stream; a tiny exp pulls the ACT table load early).
            nc.vector.memset(wrm, 0.125)
            nc.vector.memset(vp, 1.0)
            nc.vector.memset(vp2, 1.0)
            dum = consts.tile([P, 8], F16, tag="dum")
            nc.scalar.activation(
                dum, wrm[:, :8], mybir.ActivationFunctionType.Exp, bias=0.0, scale=1.0
            )
            warm = psA.tile([P, P], F32, tag="pa", name="warm")
            for i in range(16):
                nc.tensor.matmul(warm, wrm, wrm, start=(i == 0), stop=(i == 15))

            # ---------------- fill-work machinery (PE slack consumers)
            fills = deque()  # anytime work: projections, transposes, dma issues
            late = deque()  # dep-settled-late work: norm chains, E pieces

            def pop_fill(n=1):
                for _ in range(n):
                    if not fills:
                        return
                    fills.popleft()()

            # D matmuls for one pipelined unit (two heads, K=128, N=512)
            def emit_d(pd0, pd1, pt, t):
                nc.tensor.matmul(
                    pd0, vp[:, t, :], pt[:, :BLK],
                    start=(t == 0), stop=(t == NTK - 1), skip_group_check=True,
                )
                nc.tensor.matmul(
                    pd1, vp2[:, t, :], pt[:, BLK:],
                    start=(t == 0), stop=(t == NTK - 1), skip_group_check=True,
                )

            # kv projection chunk: 8 K-tiles -> kv[:, cs]; k2 bcast; transposes
            def kv_chunk_pieces(c):
                cs = slice(c * FREE, (c + 1) * FREE)
                st = {}

                def pk(i0):
                    def p():
                        if i0 == 0:
                            st["pkv"] = psA.tile([P, FREE], F32, tag="pa", name="pkv")
                        for i in range(i0, i0 + 2):
                            nc.tensor.matmul(
                                st["pkv"], wkv_sb[:, i, :], xc_t[c][:, i, :],
                                start=(i == 0), stop=(i == DT - 1),
                            )
                        if i0 == DT - 2:
                            nc.vector.tensor_copy(kv[:, cs], st["pkv"])
                            nc.sync.dma_start(k2[HEAD_DIM:, cs], kv[:HEAD_DIM, cs])

                    return p

                def p3():
                    # transpose the 4 v tiles of this chunk, batch-copy to vp/vp2
                    pvb = psA.tile([P, 4 * HEAD_DIM], F16, tag="pa", name="pvb")
                    for k in range(4):
                        ts_ = slice((4 * c + k) * P, (4 * c + k + 1) * P)
                        nc.tensor.transpose(
                            pvb[:, k * HEAD_DIM : (k + 1) * HEAD_DIM],
                            kv[HEAD_DIM:, ts_],
                            ident[HEAD_DIM:, HEAD_DIM:],
                        )
                    src = pvb.rearrange("p (k d) -> p k d", k=4)
                    nc.vector.tensor_copy(vp[:, 4 * c : 4 * c + 4, :HEAD_DIM], src)
                    nc.vector.tensor_copy(vp2[:, 4 * c : 4 * c + 4, HEAD_DIM:], src)

                return [pk(0), pk(2), pk(4), pk(6), p3]

            # q projection chunk (one e): 8 K-tiles -> qt[:, e, cs]
            def q_chunk_pieces(c, e):
                cs = slice(c * FREE, (c + 1) * FREE)
                st = {}

                def pq(i0):
                    def p():
                        if i0 == 0:
                            st["pq"] = psA.tile([P, FREE], F32, tag="pa", name="pq")
                        for i in range(i0, i0 + 2):
                            nc.tensor.matmul(
                                st["pq"], wq_sb[:, i, e * P : (e + 1) * P],
                                xq_t[c][:, i, :],
                                start=(i == 0), stop=(i == DT - 1),
                            )
                        if i0 == DT - 2:
                            nc.vector.tensor_copy(qt[:, e, cs], st["pq"])

                    return p

                return [pq(0), pq(2), pq(4), pq(6)]

            # output projection for one tq block: 8 m-tile pieces staged into
            # ySB, then half-block DMA issues.  In tail mode the py PSUM
            # accumulators rotate over 4 banks (psA pair + the pd banks,
            # which are free once the final spill ran) so the E matmuls
            # never stall on the yo casts, and stores are finer-grained so
            # the last transfer is small.
            def e_pieces(blk, tail=False):
                bs = slice(blk * BLK, (blk + 1) * BLK)
                ysb = ypool.tile([P, MT, FREE], F16, tag="ysb", name=f"ysb{blk}")

                def mk(m):
                    def p():
                        if tail and m % 2 == 1:
                            tag = "pd0" if m % 4 == 1 else "pd1"
                            py = psD.tile([P, FREE], F32, tag=tag, name="py")
                        else:
                            py = psA.tile([P, FREE], F32, tag="pa", name="py")
                        for ee in range(ET):
                            nc.tensor.matmul(
                                py, wo_sb[:, ee, m * P : (m + 1) * P], outs[:, ee, bs],
                                start=(ee == 0), stop=(ee == ET - 1),
                            )
                        nc.vector.tensor_copy(ysb[:, m, :], py)

                    return p

                def store(h, nst):
                    def p():
                        ms = slice(h * (MT // nst), (h + 1) * (MT // nst))
                        nc.sync.dma_start(yt[:, blk, ms, :], ysb[:, ms, :])

                    return p

                pieces = [mk(m) for m in range(MT)]
                if tail:
                    out = []
                    for m in range(MT):
                        out.append(pieces[m])
                        if m % 2 == 1:
                            out.append(store(m // 2, 4))
                    return out
                return pieces + [store(0, 2), store(1, 2)]

            # spill pd0/pd1 for one section into aligned full-partition
            # tiles: rawN = [AV_even | AV_odd], rawD = [den_odd | den_even]
            # (reciprocal_approx_fast silently corrupts partition-offset
            # operands, so the custom op must see full offset-0 tiles).
            def spill(dp0, dp1):
                rawN = recpool.tile([P, BLK], F32, tag="rawN", name="rawN")
                rawD = recpool.tile([P, BLK], F32, tag="rawD", name="rawD")
                nc.vector.tensor_copy(rawN[:HEAD_DIM, :], dp0[:HEAD_DIM, :])
                nc.vector.tensor_copy(rawN[HEAD_DIM:, :], dp1[HEAD_DIM:, :])
                nc.vector.tensor_copy(rawD[:HEAD_DIM, :], dp1[:HEAD_DIM, :])
                nc.vector.tensor_copy(rawD[HEAD_DIM:, :], dp0[HEAD_DIM:, :])
                return rawN, rawD

            # normalize chain for one section (deferred into the late window
            # of the following section): one full-partition fast recip, two
            # half-swap broadcast DMAs, one full-width multiply.
            def norm_chain(sec, rawN, rawD):
                blk, e = divmod(sec, ET)
                bs = slice(blk * BLK, (blk + 1) * BLK)

                def p():
                    recD = recpool.tile([P, BLK], F32, tag="recD", name="recD")
                    recS = recpool.tile([P, BLK], F32, tag="recS", name="recS")
                    nc.vector.reciprocal_approx_fast(recD, rawD)
                    nc.sync.dma_start(recS[:HEAD_DIM, :], recD[HEAD_DIM:, :])
                    nc.sync.dma_start(recS[HEAD_DIM:, :], recD[:HEAD_DIM, :])
                    nc.vector.tensor_mul(outs[:, e, bs], rawN, recS)

                return p

            # ---------------- lead-in PE work: kv chunk 0 + q chunk 0 (e=0)
            # inline; everything else is fills with deadline-ordered layout.
            kc0 = kv_chunk_pieces(0)
            for piece in kc0[:4]:
                piece()
            for piece in q_chunk_pieces(0, 0):
                piece()

            # Fill deadline order for section 0 (2 pops/unit):
            #   kc0.p3 (v transposes, before D(0) at u=2) -> t<=1
            #   kc1 pk pieces before B(4) -> t<=3 ; kc1.p3 before D(4) -> t<=5
            #   kc2 before B(8); kc3 before B(12); q(0,1) before section 1.
            kcs = [None, kv_chunk_pieces(1), kv_chunk_pieces(2), kv_chunk_pieces(3)]
            fills.append(kc0[4])
            fills.extend(kcs[1][:4])
            fills.append(kcs[1][4])
            fills.extend(kcs[2][:5])
            fills.extend(kcs[3][:5])
            fills.extend(q_chunk_pieces(0, 1))
            for e in range(ET):
                fills.extend(q_chunk_pieces(1, e))
            for e in range(ET):
                fills.extend(q_chunk_pieces(2, e))
            for e in range(ET):
                fills.extend(q_chunk_pieces(3, e))

            # ---------------- the continuous BCD pipeline over 128 units
            units = [(sec, t) for sec in range(NSEC) for t in range(NTK)]
            pending = deque()  # (pd0, pd1, pt, t, sec)
            pd_cur = None
            for u, (sec, t) in enumerate(units):
                blk, e = divmod(sec, ET)
                bs = slice(blk * BLK, (blk + 1) * BLK)
                if t == 0:
                    pd_cur = (
                        psD.tile([P, BLK], F32, tag="pd0", name="pd0"),
                        psD.tile([P, BLK], F32, tag="pd1", name="pd1"),
                    )
                pb = psS.tile([P, 2 * BLK], F32, tag="pb", name="pb")
                # B: two K=64 row-group matmuls, concurrent in the array
                nc.tensor.matmul(pb[:, :BLK], kv[:HEAD_DIM, t * P : (t + 1) * P],
                                 qt[:HEAD_DIM, e, bs])
                nc.tensor.matmul(pb[:, BLK:], k2[HEAD_DIM:, t * P : (t + 1) * P],
                                 qt[HEAD_DIM:, e, bs])
                if len(pending) >= 2:
                    dp0, dp1, dpt, dt_, dsec = pending.popleft()
                    emit_d(dp0, dp1, dpt, dt_)
                    if dt_ == NTK - 1 and dsec < NSEC - 1:
                        # section dsec fully accumulated: spill pd -> raw
                        # (frees the PSUM banks for this section's own Ds),
                        # queue the normalize + block-complete E work.
                        rawN, rawD = spill(dp0, dp1)
                        late.append(norm_chain(dsec, rawN, rawD))
                        if dsec % ET == ET - 1 and dsec >= 1:
                            late.extend(e_pieces(dsec // ET))
                pt = ptpool.tile([P, 2 * BLK], F16, tag="pt", name="pt")
                nc.scalar.activation(
                    pt, pb, mybir.ActivationFunctionType.Exp, bias=0.0, scale=SCALE
                )
                pending.append((pd_cur[0], pd_cur[1], pt, t, sec))
                if sec == 0:
                    pop_fill(2)
                elif 5 <= t <= 14 and late:
                    late.popleft()()
                elif t >= 2:
                    pop_fill(1)

            # ---------------- tail: drain last two Ds, normalize the final
            # section straight out of PSUM (fast recip), output-project the
            # last block, store.
            final_pd = None
            while pending:
                dp0, dp1, dpt, dt_, dsec = pending.popleft()
                emit_d(dp0, dp1, dpt, dt_)
                final_pd = (dp0, dp1)
            while late:
                late.popleft()()
            rawN, rawD = spill(final_pd[0], final_pd[1])
            norm_chain(NSEC - 1, rawN, rawD)()
            for piece in e_pieces(NBLK - 1, tail=True):
                piece()
            while fills:
                pop_fill()

    nc.finalize()
    return nc


_NC_CACHE = None


def _get_nc():
    global _NC_CACHE
    if _NC_CACHE is None:
        _NC_CACHE = build_bass()
    return _NC_CACHE


def _cid():
    return np.eye(P, dtype=np.float16)


def _chunked(xT):
    """[D_MODEL, T] -> [P, NCH, DT, FREE] with row i*P+p at [p, :, i, :]:
    each partition's chunk data contiguous for large-descriptor DMA."""
    return np.ascontiguousarray(
        xT.reshape(DT, P, NCH, FREE).transpose(1, 2, 0, 3)
    ).astype(np.float16)


def _wtiles(wT):
    """[D_MODEL, E] -> [P, DT, E]"""
    return np.ascontiguousarray(
        wT.reshape(DT, P, wT.shape[1]).transpose(1, 0, 2)
    ).astype(np.float16)


def shard_inputs(query, context, Wq, Wk, Wv, Wo):
    """host-side sharding: 8 cores = batch(2) x kv-group(4)"""
    in_maps = []
    xqh = [_chunked(np.asarray(query[b]).T) for b in range(B)]
    xch = [_chunked(np.asarray(context[b]).T) for b in range(B)]
    for core in range(N_CORES):
        b, g = divmod(core, GROUPS)
        wqh = _wtiles(Wq[g * DQ : (g + 1) * DQ, :].T)
        wkvh = _wtiles(
            np.concatenate(
                [
                    Wk[g * HEAD_DIM : (g + 1) * HEAD_DIM, :],
                    Wv[g * HEAD_DIM : (g + 1) * HEAD_DIM, :],
                ],
                axis=0,
            ).T
        )
        woT = Wo[:, g * DQ : (g + 1) * DQ].T  # [DQ, D_MODEL]
        woh = np.ascontiguousarray(
            woT.reshape(ET, P, D_MODEL).transpose(1, 0, 2)
        ).astype(np.float16)
        in_maps.append(
            {
                "xqh": xqh[b],
                "xch": xch[b],
                "wqh": wqh,
                "wkvh": wkvh,
                "woh": woh,
                "cid": _cid(),
            }
        )
    return in_maps


def kernel(query, context, Wq, Wk, Wv, Wo, _want_profile=False):
    from concourse.bass_utils import run_bass_kernel_spmd

    nc = _get_nc()
    in_maps = shard_inputs(query, context, Wq, Wk, Wv, Wo)
    res = run_bass_kernel_spmd(
        nc, in_maps, core_ids=list(range(N_CORES)), trace=_want_profile
    )
    out = np.zeros((B, TQ, D_MODEL), dtype=np.float32)
    for core in range(N_CORES):
        b = core // GROUPS
        yh = res.results[core]["yh"].astype(np.float32)
        yT = yh.transpose(2, 0, 1, 3).reshape(D_MODEL, TQ)
        out[b] += yT.T
    if _want_profile:
        return out, res
    return out


# revision 18
# speedup vs baseline: 1.1640x; 1.0090x over previous
"""Cross-attention (GQA) Trainium2 Bass kernel — pipelined v3.

Problem: B=2, Tq=Tkv=2048, D_MODEL=1024, 16 query heads / 4 kv heads,
head_dim=64.  Sharded over 8 NeuronCores as batch(2) x kv-group(4); each
core computes 4 query heads + its single kv head and a partial output
projection (Wo row-split by head group); partials are summed on host.

Dataflow (feature dim on SBUF partitions end-to-end, no big transposes):

  A: qT[e,t] = WqT.T @ xqT,  kvT = WkvT.T @ xcT   (weights stationary)
     v[tk,dv] via PE-transpose of vT tiles; vp=[v|1], vp2=[1|v]
  B: per (sec=blk,e) unit t: pb[128,1024] = two K=64 row-group matmuls
     (h_even rows 0-63 -> cols 0:512, h_odd rows 64-127 -> cols
     512:1024), concurrent in the PE array.
  C: pt = exp(pb/8) one ScalarE instruction per unit (FD=1024).
  D: pd_h[128,512] += vp_t.T @ pt_half; ones-columns give the softmax
     denominators in the complementary 64 partitions.
  E: yT += WoT.T @ (pd*recip(den)), row-split by head pair.

v3 over v2:
  - ONE continuous 128-unit software pipeline (B(u), D(u-2), exp(u))
    across all 8 sections: no PE drain / exp restart bubble at section
    boundaries.
  - reciprocal_approx_fast (custom DVE op, ~5x faster than RECIPROCAL)
    in the softmax-denominator normalize chains.
  - Input DMAs split across BOTH hardware DGE queues: xq/wq/wo/ident
    issue from the (idle during lead-in) scalar/Act queue, xc/wkv from
    the sync queue, all unconditionally at kernel start so no waiting
    DMA ever sits in front of an input load.
  - Norm chains and output-projection (E) pieces pop from a dedicated
    "late" queue only at units t in [5..14] of the FOLLOWING section,
    when their DVE-side producers have long retired - they never stall
    the in-order PE.
  - y stores staged per tq-block in SBUF and written with 2 half-block
    DMAs (8 DMA issues/block -> 2), keeping the sync queue uncongested
    so the rec-broadcast DMAs on the norm critical path fire promptly.

ScalarE (the 1 elem/cycle/lane exp bottleneck, ~143us) paces the
kernel; the PE stays dense and HAM-warm.
"""

import os
import sys
from collections import deque

import numpy as np

for _p in ("/opt/trn_rl_repo",):
    if _p not in sys.path and os.path.isdir(_p):
        sys.path.insert(0, _p)

import concourse.bass as bass
import concourse.bacc as bacc
import concourse.mybir as mybir
from concourse.tile import TileContext

# ---------------------------------------------------------------- problem dims
B = 2
TQ = 2048
TKV = 2048
D_MODEL = 1024
N_HEADS = 16
N_KV_HEADS = 4
HEAD_DIM = 64
N_CORES = 8
GROUPS = N_KV_HEADS  # kv groups = 4
HEADS_PER_DEV = N_HEADS // GROUPS  # 4
DQ = HEADS_PER_DEV * HEAD_DIM  # 256
DKV = 2 * HEAD_DIM  # 128 (k rows + v rows stacked)
SCALE = 1.0 / float(np.sqrt(HEAD_DIM))

P = 128
FREE = 512  # matmul moving-operand chunk / tq block width
BLK = 512
NBLK = TQ // BLK  # 4 tq blocks
DT = D_MODEL // P  # 8 d-tiles
ET = DQ // P  # 2 e-tiles (query head pairs)
NCH = TQ // FREE  # 4 x chunks of 512
NTK = TKV // P  # 16 tk tiles
MT = D_MODEL // P  # 8 output m-tiles
NSEC = NBLK * ET  # 8 sections
NU = NSEC * NTK  # 128 pipelined units

F32 = mybir.dt.float32
F16 = mybir.dt.float16


def build_bass():
    nc = bacc.Bacc()

    # all inputs/outputs are host-pre-arranged to be contiguous per SBUF
    # partition: each load/store is ~128 large descriptors, not 1024 small
    # ones (DGE issue cost and HBM efficiency both scale with that).
    xq = nc.declare_dram_parameter("xqh", [P, NCH, DT, FREE], F16, isOutput=False)
    xc = nc.declare_dram_parameter("xch", [P, NCH, DT, FREE], F16, isOutput=False)
    wq = nc.declare_dram_parameter("wqh", [P, DT, DQ], F16, isOutput=False)
    wkv = nc.declare_dram_parameter("wkvh", [P, DT, DKV], F16, isOutput=False)
    wo = nc.declare_dram_parameter("woh", [P, ET, D_MODEL], F16, isOutput=False)
    # block-swap matrix [[0,I64],[I64,0]]: cid2[64:, :64] is a plain I64 for
    # PE transposes; the full matrix PE-shifts kT from partitions 0-63 into
    # 64-127 (replacing a scheduler-hostile SBUF->SBUF broadcast DMA).
    cid = nc.declare_dram_parameter("cid2", [P, P], F16, isOutput=False)
    yt = nc.declare_dram_parameter("yh", [P, NBLK, MT, FREE], F16, isOutput=True)

    with TileContext(nc) as tc:
        with (
            tc.tile_pool(name="consts", bufs=1) as consts,
            tc.tile_pool(name="pt", bufs=4) as ptpool,
            tc.tile_pool(name="rec", bufs=2) as recpool,
            tc.tile_pool(name="yout", bufs=2) as ypool,
            tc.tile_pool(name="psS", bufs=2, space="PSUM") as psS,
            tc.tile_pool(name="psD", bufs=1, space="PSUM") as psD,
            tc.tile_pool(name="psA", bufs=2, space="PSUM") as psA,
        ):
            # ---------------- persistent tiles
            qt = consts.tile([P, ET, TQ], F16, tag="qt")  # head pair per e
            kv = consts.tile([P, TKV], F16, tag="kv")  # rows 0-63 kT, 64-127 vT
            k2 = consts.tile([P, TKV], F16, tag="k2")  # rows 64-127 = kT copy
            vp = consts.tile([P, NTK, P], F16, tag="vp")  # [v | ones]
            vp2 = consts.tile([P, NTK, P], F16, tag="vp2")  # [ones | v]
            outs = consts.tile([P, ET, TQ], F16, tag="outs")  # normalized outT
            ident = consts.tile([P, P], F16, tag="ident")
            wkv_sb = consts.tile([P, DT, DKV], F16, tag="wkv")
            wq_sb = consts.tile([P, DT, DQ], F16, tag="wq")
            wo_sb = consts.tile([P, ET, D_MODEL], F16, tag="wo")
            wrm = consts.tile([P, P], F16, tag="wrm")
            xq_t = [
                consts.tile([P, DT, FREE], F16, tag=f"xq{c}", name=f"xq{c}")
                for c in range(NCH)
            ]
            xc_t = [
                consts.tile([P, DT, FREE], F16, tag=f"xc{c}", name=f"xc{c}")
                for c in range(NCH)
            ]

            # ---------------- input DMA burst: everything issues up front,
            # split across the two hardware DGE queues.  Neither queue has a
            # waiting DMA in front of an input load.  Most-urgent first.
            # sync queue: kv path (xc chunks feed the B-matmul deadline
            # chain), then the non-urgent q-path chunks.
            nc.sync.dma_start(wkv_sb, wkv[:])
            for c in range(NCH):
                nc.sync.dma_start(xc_t[c], xc[:, c, :, :])
            for c in range(1, NCH):
                nc.sync.dma_start(xq_t[c], xq[:, c, :, :])
            nc.sync.dma_start(wo_sb, wo[:])
            # scalar/Act queue (idle until the first exp): only what the
            # first B matmul needs, so the issue cost never delays exp(0).
            nc.scalar.dma_start(xq_t[0], xq[:, 0, :, :])
            nc.scalar.dma_start(wq_sb, wq[:])
            nc.scalar.dma_start(ident, cid[:])

            # vector-side const init + HAM warm-up (no DMA dependencies:
            # wrm memset feeds dummy matmuls that spin the PE clock up while
            # the inputs stream; a tiny exp pulls the ACT table load early).
            nc.vector.memset(wrm, 0.125)
            nc.vector.memset(vp, 1.0)
            nc.vector.memset(vp2, 1.0)
            dum = consts.tile([P, 8], F16, tag="dum")
            nc.scalar.activation(
                dum, wrm[:, :8], mybir.ActivationFunctionType.Exp, bias=0.0, scale=1.0
            )
            warm = psA.tile([P, P], F32, tag="pa", name="warm")
            for i in range(16):
                nc.tensor.matmul(warm, wrm, wrm, start=(i == 0), stop=(i == 15))

            # ---------------- fill-work machinery (PE slack consumers)
            fills = deque()  # anytime work: projections, transposes, dma issues
            late = deque()  # dep-settled-late work: norm chains, E pieces

            def pop_fill(n=1):
                for _ in range(n):
                    if not fills:
                        return
                    fills.popleft()()

            # D matmuls for one pipelined unit (two heads, K=128, N=512)
            def emit_d(pd0, pd1, pt, t):
                nc.tensor.matmul(
                    pd0, vp[:, t, :], pt[:, :BLK],
                    start=(t == 0), stop=(t == NTK - 1), skip_group_check=True,
                )
                nc.tensor.matmul(
                    pd1, vp2[:, t, :], pt[:, BLK:],
                    start=(t == 0), stop=(t == NTK - 1), skip_group_check=True,
                )

            # kv projection chunk: 8 K-tiles -> kv[:, cs]; k2 shift; transposes
            def kv_chunk_pieces(c):
                cs = slice(c * FREE, (c + 1) * FREE)
                st = {}

                def pk(i0):
                    def p():
                        if i0 == 0:
                            st["pkv"] = psA.tile([P, FREE], F32, tag="pa", name="pkv")
                        for i in range(i0, i0 + 2):
                            nc.tensor.matmul(
                                st["pkv"], wkv_sb[:, i, :], xc_t[c][:, i, :],
                                start=(i == 0), stop=(i == DT - 1),
                            )
                        if i0 == DT - 2:
                            nc.vector.tensor_copy(kv[:, cs], st["pkv"])

                    return p

                def p4():
                    # kT -> partitions 64-127 of k2 via the block-swap matmul
                    psK = psA.tile([P, FREE], F32, tag="pa", name="psK")
                    nc.tensor.matmul(psK, ident[:HEAD_DIM, :], kv[:HEAD_DIM, cs])
                    nc.vector.tensor_copy(k2[HEAD_DIM:, cs], psK[HEAD_DIM:, :])

                def p3():
                    # transpose the 4 v tiles of this chunk, batch-copy to vp/vp2
                    pvb = psA.tile([P, 4 * HEAD_DIM], F16, tag="pa", name="pvb")
                    for k in range(4):
                        ts_ = slice((4 * c + k) * P, (4 * c + k + 1) * P)
                        nc.tensor.transpose(
                            pvb[:, k * HEAD_DIM : (k + 1) * HEAD_DIM],
                            kv[HEAD_DIM:, ts_],
                            ident[HEAD_DIM:, :HEAD_DIM],
                        )
                    src = pvb.rearrange("p (k d) -> p k d", k=4)
                    nc.vector.tensor_copy(vp[:, 4 * c : 4 * c + 4, :HEAD_DIM], src)
                    nc.vector.tensor_copy(vp2[:, 4 * c : 4 * c + 4, HEAD_DIM:], src)

                return [pk(0), pk(2), pk(4), pk(6), p4, p3]

            # q projection chunk (one e): 8 K-tiles -> qt[:, e, cs]
            def q_chunk_pieces(c, e):
                cs = slice(c * FREE, (c + 1) * FREE)
                st = {}

                def pq(i0):
                    def p():
                        if i0 == 0:
                            st["pq"] = psA.tile([P, FREE], F32, tag="pa", name="pq")
                        for i in range(i0, i0 + 2):
                            nc.tensor.matmul(
                                st["pq"], wq_sb[:, i, e * P : (e + 1) * P],
                                xq_t[c][:, i, :],
                                start=(i == 0), stop=(i == DT - 1),
                            )
                        if i0 == DT - 2:
                            nc.vector.tensor_copy(qt[:, e, cs], st["pq"])

                    return p

                return [pq(0), pq(2), pq(4), pq(6)]

            # output projection for one tq block: 8 m-tile pieces staged into
            # ySB, then half-block DMA issues.  In tail mode the py PSUM
            # accumulators rotate over 4 banks (psA pair + the pd banks,
            # which are free once the final spill ran) so the E matmuls
            # never stall on the yo casts, and stores are finer-grained so
            # the last transfer is small.
            def e_pieces(blk, tail=False):
                bs = slice(blk * BLK, (blk + 1) * BLK)
                ysb = ypool.tile([P, MT, FREE], F16, tag="ysb", name=f"ysb{blk}")

                def mk(m):
                    def p():
                        if tail and m % 2 == 1:
                            tag = "pd0" if m % 4 == 1 else "pd1"
                            py = psD.tile([P, FREE], F32, tag=tag, name="py")
                        else:
                            py = psA.tile([P, FREE], F32, tag="pa", name="py")
                        for ee in range(ET):
                            nc.tensor.matmul(
                                py, wo_sb[:, ee, m * P : (m + 1) * P], outs[:, ee, bs],
                                start=(ee == 0), stop=(ee == ET - 1),
                            )
                        nc.vector.tensor_copy(ysb[:, m, :], py)

                    return p

                def store(h, nst):
                    def p():
                        ms = slice(h * (MT // nst), (h + 1) * (MT // nst))
                        nc.sync.dma_start(yt[:, blk, ms, :], ysb[:, ms, :])

                    return p

                pieces = [mk(m) for m in range(MT)]
                if tail:
                    out = []
                    for m in range(MT):
                        out.append(pieces[m])
                        if m % 2 == 1:
                            out.append(store(m // 2, 4))
                    return out
                return pieces + [store(0, 2), store(1, 2)]

            # spill pd0/pd1 for one section into aligned full-partition
            # tiles: rawN = [AV_even | AV_odd], rawD = [den_odd | den_even]
            # (reciprocal_approx_fast silently corrupts partition-offset
            # operands, so the custom op must see full offset-0 tiles).
            def spill(dp0, dp1):
                rawN = recpool.tile([P, BLK], F32, tag="rawN", name="rawN")
                rawD = recpool.tile([P, BLK], F32, tag="rawD", name="rawD")
                nc.vector.tensor_copy(rawN[:HEAD_DIM, :], dp0[:HEAD_DIM, :])
                nc.vector.tensor_copy(rawN[HEAD_DIM:, :], dp1[HEAD_DIM:, :])
                nc.vector.tensor_copy(rawD[:HEAD_DIM, :], dp1[:HEAD_DIM, :])
                nc.vector.tensor_copy(rawD[HEAD_DIM:, :], dp0[HEAD_DIM:, :])
                return rawN, rawD

            # normalize chain for one section (deferred into the late window
            # of the following section): one full-partition fast recip, two
            # half-swap broadcast DMAs, one full-width multiply.
            def norm_chain(sec, rawN, rawD):
                blk, e = divmod(sec, ET)
                bs = slice(blk * BLK, (blk + 1) * BLK)

                def p():
                    recD = recpool.tile([P, BLK], F32, tag="recD", name="recD")
                    recS = recpool.tile([P, BLK], F32, tag="recS", name="recS")
                    nc.vector.reciprocal_approx_fast(recD, rawD)
                    nc.sync.dma_start(recS[:HEAD_DIM, :], recD[HEAD_DIM:, :])
                    nc.sync.dma_start(recS[HEAD_DIM:, :], recD[:HEAD_DIM, :])
                    nc.vector.tensor_mul(outs[:, e, bs], rawN, recS)

                return p

            # ---------------- lead-in PE work: kv chunk 0 + q chunk 0 (e=0)
            # inline; everything else is fills with deadline-ordered layout.
            kc0 = kv_chunk_pieces(0)
            for piece in kc0[:4]:
                piece()
            for piece in q_chunk_pieces(0, 0):
                piece()
            kc0[4]()  # k2 shift for chunk 0 (B(0) reads it)

            # Fill deadline order for section 0 (2 pops/unit):
            #   kc0.p3 (v transposes, before D(0) at u=2) -> t=0
            #   kc{c}: pk x4 + p4 (k2 shift) before B(4c); p3 before D(4c).
            fills.append(kc0[5])
            fills.extend(kv_chunk_pieces(1))
            fills.extend(kv_chunk_pieces(2))
            fills.extend(kv_chunk_pieces(3))
            fills.extend(q_chunk_pieces(0, 1))
            for e in range(ET):
                fills.extend(q_chunk_pieces(1, e))
            for e in range(ET):
                fills.extend(q_chunk_pieces(2, e))
            for e in range(ET):
                fills.extend(q_chunk_pieces(3, e))

            # ---------------- the continuous BCD pipeline over 128 units
            units = [(sec, t) for sec in range(NSEC) for t in range(NTK)]
            pending = deque()  # (pd0, pd1, pt, t, sec)
            pd_cur = None
            for u, (sec, t) in enumerate(units):
                blk, e = divmod(sec, ET)
                bs = slice(blk * BLK, (blk + 1) * BLK)
                if t == 0:
                    pd_cur = (
                        psD.tile([P, BLK], F32, tag="pd0", name="pd0"),
                        psD.tile([P, BLK], F32, tag="pd1", name="pd1"),
                    )
                pb = psS.tile([P, 2 * BLK], F32, tag="pb", name="pb")
                # B: two K=64 row-group matmuls, concurrent in the array
                nc.tensor.matmul(pb[:, :BLK], kv[:HEAD_DIM, t * P : (t + 1) * P],
                                 qt[:HEAD_DIM, e, bs])
                nc.tensor.matmul(pb[:, BLK:], k2[HEAD_DIM:, t * P : (t + 1) * P],
                                 qt[HEAD_DIM:, e, bs])
                if len(pending) >= 2:
                    dp0, dp1, dpt, dt_, dsec = pending.popleft()
                    emit_d(dp0, dp1, dpt, dt_)
                    if dt_ == NTK - 1 and dsec < NSEC - 1:
                        # section dsec fully accumulated: spill pd -> raw
                        # (frees the PSUM banks for this section's own Ds),
                        # queue the normalize + block-complete E work.
                        rawN, rawD = spill(dp0, dp1)
                        late.append(norm_chain(dsec, rawN, rawD))
                        if dsec % ET == ET - 1 and dsec >= 1:
                            late.extend(e_pieces(dsec // ET))
                pt = ptpool.tile([P, 2 * BLK], F16, tag="pt", name="pt")
                nc.scalar.activation(
                    pt, pb, mybir.ActivationFunctionType.Exp, bias=0.0, scale=SCALE
                )
                pending.append((pd_cur[0], pd_cur[1], pt, t, sec))
                if sec == 0:
                    pop_fill(2)
                elif 5 <= t <= 14 and late:
                    late.popleft()()
                elif t >= 2:
                    pop_fill(1)

            # ---------------- tail: drain last two Ds, normalize the final
            # section straight out of PSUM (fast recip), output-project the
            # last block, store.
            final_pd = None
            while pending:
                dp0, dp1, dpt, dt_, dsec = pending.popleft()
                emit_d(dp0, dp1, dpt, dt_)
                final_pd = (dp0, dp1)
            while late:
                late.popleft()()
            rawN, rawD = spill(final_pd[0], final_pd[1])
            norm_chain(NSEC - 1, rawN, rawD)()
            for piece in e_pieces(NBLK - 1, tail=True):
                piece()
            while fills:
                pop_fill()

    nc.finalize()
    return nc


_NC_CACHE = None


def _get_nc():
    global _NC_CACHE
    if _NC_CACHE is None:
        _NC_CACHE = build_bass()
    return _NC_CACHE


def _cid2():
    z = np.zeros((HEAD_DIM, HEAD_DIM), dtype=np.float16)
    i = np.eye(HEAD_DIM, dtype=np.float16)
    return np.block([[z, i], [i, z]])


def _chunked(xT):
    """[D_MODEL, T] -> [P, NCH, DT, FREE] with row i*P+p at [p, :, i, :]:
    each partition's chunk data contiguous for large-descriptor DMA."""
    return np.ascontiguousarray(
        xT.reshape(DT, P, NCH, FREE).transpose(1, 2, 0, 3)
    ).astype(np.float16)


def _wtiles(wT):
    """[D_MODEL, E] -> [P, DT, E]"""
    return np.ascontiguousarray(
        wT.reshape(DT, P, wT.shape[1]).transpose(1, 0, 2)
    ).astype(np.float16)


def shard_inputs(query, context, Wq, Wk, Wv, Wo):
    """host-side sharding: 8 cores = batch(2) x kv-group(4)"""
    in_maps = []
    xqh = [_chunked(np.asarray(query[b]).T) for b in range(B)]
    xch = [_chunked(np.asarray(context[b]).T) for b in range(B)]
    for core in range(N_CORES):
        b, g = divmod(core, GROUPS)
        wqh = _wtiles(Wq[g * DQ : (g + 1) * DQ, :].T)
        wkvh = _wtiles(
            np.concatenate(
                [
                    Wk[g * HEAD_DIM : (g + 1) * HEAD_DIM, :],
                    Wv[g * HEAD_DIM : (g + 1) * HEAD_DIM, :],
                ],
                axis=0,
            ).T
        )
        woT = Wo[:, g * DQ : (g + 1) * DQ].T  # [DQ, D_MODEL]
        woh = np.ascontiguousarray(
            woT.reshape(ET, P, D_MODEL).transpose(1, 0, 2)
        ).astype(np.float16)
        in_maps.append(
            {
                "xqh": xqh[b],
                "xch": xch[b],
                "wqh": wqh,
                "wkvh": wkvh,
                "woh": woh,
                "cid2": _cid2(),
            }
        )
    return in_maps


def kernel(query, context, Wq, Wk, Wv, Wo, _want_profile=False):
    from concourse.bass_utils import run_bass_kernel_spmd

    nc = _get_nc()
    in_maps = shard_inputs(query, context, Wq, Wk, Wv, Wo)
    res = run_bass_kernel_spmd(
        nc, in_maps, core_ids=list(range(N_CORES)), trace=_want_profile
    )
    out = np.zeros((B, TQ, D_MODEL), dtype=np.float32)
    for core in range(N_CORES):
        b = core // GROUPS
        yh = res.results[core]["yh"].astype(np.float32)
        yT = yh.transpose(2, 0, 1, 3).reshape(D_MODEL, TQ)
        out[b] += yT.T
    if _want_profile:
        return out, res
    return out


# revision 23
# speedup vs baseline: 1.1898x; 1.0222x over previous
"""Cross-attention (GQA) Trainium2 Bass kernel — pipelined v3.

Problem: B=2, Tq=Tkv=2048, D_MODEL=1024, 16 query heads / 4 kv heads,
head_dim=64.  Sharded over 8 NeuronCores as batch(2) x kv-group(4); each
core computes 4 query heads + its single kv head and a partial output
projection (Wo row-split by head group); partials are summed on host.

Dataflow (feature dim on SBUF partitions end-to-end, no big transposes):

  A: qT[e,t] = WqT.T @ xqT,  kvT = WkvT.T @ xcT   (weights stationary)
     v[tk,dv] via PE-transpose of vT tiles; vp=[v|1], vp2=[1|v]
  B: per (sec=blk,e) unit t: pb[128,1024] = two K=64 row-group matmuls
     (h_even rows 0-63 -> cols 0:512, h_odd rows 64-127 -> cols
     512:1024), concurrent in the PE array.
  C: pt = exp(pb/8) one ScalarE instruction per unit (FD=1024).
  D: pd_h[128,512] += vp_t.T @ pt_half; ones-columns give the softmax
     denominators in the complementary 64 partitions.
  E: yT += WoT.T @ (pd*recip(den)), row-split by head pair.

v3 over v2:
  - ONE continuous 128-unit software pipeline (B(u), D(u-2), exp(u))
    across all 8 sections: no PE drain / exp restart bubble at section
    boundaries.
  - reciprocal_approx_fast (custom DVE op, ~5x faster than RECIPROCAL)
    in the softmax-denominator normalize chains.
  - Input DMAs split across BOTH hardware DGE queues: xq/wq/wo/ident
    issue from the (idle during lead-in) scalar/Act queue, xc/wkv from
    the sync queue, all unconditionally at kernel start so no waiting
    DMA ever sits in front of an input load.
  - Norm chains and output-projection (E) pieces pop from a dedicated
    "late" queue only at units t in [5..14] of the FOLLOWING section,
    when their DVE-side producers have long retired - they never stall
    the in-order PE.
  - y stores staged per tq-block in SBUF and written with 2 half-block
    DMAs (8 DMA issues/block -> 2), keeping the sync queue uncongested
    so the rec-broadcast DMAs on the norm critical path fire promptly.

ScalarE (the 1 elem/cycle/lane exp bottleneck, ~143us) paces the
kernel; the PE stays dense and HAM-warm.
"""

import os
import sys
from collections import deque

import numpy as np

for _p in ("/opt/trn_rl_repo",):
    if _p not in sys.path and os.path.isdir(_p):
        sys.path.insert(0, _p)

import concourse.bass as bass
import concourse.bacc as bacc
import concourse.mybir as mybir
from concourse.tile import TileContext

# ---------------------------------------------------------------- problem dims
B = 2
TQ = 2048
TKV = 2048
D_MODEL = 1024
N_HEADS = 16
N_KV_HEADS = 4
HEAD_DIM = 64
N_CORES = 8
GROUPS = N_KV_HEADS  # kv groups = 4
HEADS_PER_DEV = N_HEADS // GROUPS  # 4
DQ = HEADS_PER_DEV * HEAD_DIM  # 256
DKV = 2 * HEAD_DIM  # 128 (k rows + v rows stacked)
SCALE = 1.0 / float(np.sqrt(HEAD_DIM))

P = 128
FREE = 512  # matmul moving-operand chunk / tq block width
BLK = 512
NBLK = TQ // BLK  # 4 tq blocks
DT = D_MODEL // P  # 8 d-tiles
ET = DQ // P  # 2 e-tiles (query head pairs)
NCH = TQ // FREE  # 4 x chunks of 512
NTK = TKV // P  # 16 tk tiles
MT = D_MODEL // P  # 8 output m-tiles
NSEC = NBLK * ET  # 8 sections
NU = NSEC * NTK  # 128 pipelined units

F32 = mybir.dt.float32
F16 = mybir.dt.float16


def build_bass():
    nc = bacc.Bacc()

    # all inputs/outputs are host-pre-arranged to be contiguous per SBUF
    # partition: each load/store is ~128 large descriptors, not 1024 small
    # ones (DGE issue cost and HBM efficiency both scale with that).
    xq = nc.declare_dram_parameter("xqh", [P, NCH, DT, FREE], F16, isOutput=False)
    xc = nc.declare_dram_parameter("xch", [P, NCH, DT, FREE], F16, isOutput=False)
    wq = nc.declare_dram_parameter("wqh", [P, DT, DQ], F16, isOutput=False)
    wkv = nc.declare_dram_parameter("wkvh", [P, DT, DKV], F16, isOutput=False)
    wo = nc.declare_dram_parameter("woh", [P, ET, D_MODEL], F16, isOutput=False)
    # block-swap matrix [[0,I64],[I64,0]]: cid2[64:, :64] is a plain I64 for
    # PE transposes; the full matrix PE-shifts kT from partitions 0-63 into
    # 64-127 (replacing a scheduler-hostile SBUF->SBUF broadcast DMA).
    cid = nc.declare_dram_parameter("cid2", [P, P], F16, isOutput=False)
    yt = nc.declare_dram_parameter("yh", [P, NBLK, MT, FREE], F16, isOutput=True)

    with TileContext(nc) as tc:
        with (
            tc.tile_pool(name="consts", bufs=1) as consts,
            tc.tile_pool(name="pt", bufs=4) as ptpool,
            tc.tile_pool(name="rec", bufs=2) as recpool,
            tc.tile_pool(name="yout", bufs=2) as ypool,
            tc.tile_pool(name="psS", bufs=2, space="PSUM") as psS,
            tc.tile_pool(name="psD", bufs=1, space="PSUM") as psD,
            tc.tile_pool(name="psA", bufs=2, space="PSUM") as psA,
        ):
            # ---------------- persistent tiles
            qt = consts.tile([P, ET, TQ], F16, tag="qt")  # head pair per e
            kv = consts.tile([P, TKV], F16, tag="kv")  # rows 0-63 kT, 64-127 vT
            k2 = consts.tile([P, TKV], F16, tag="k2")  # rows 64-127 = kT copy
            vp = consts.tile([P, NTK, P], F16, tag="vp")  # [v | ones]
            vp2 = consts.tile([P, NTK, P], F16, tag="vp2")  # [ones | v]
            outs = consts.tile([P, ET, TQ], F16, tag="outs")  # normalized outT
            ident = consts.tile([P, P], F16, tag="ident")
            wkv_sb = consts.tile([P, DT, DKV], F16, tag="wkv")
            wq_sb = consts.tile([P, DT, DQ], F16, tag="wq")
            wo_sb = consts.tile([P, ET, D_MODEL], F16, tag="wo")
            wrm = consts.tile([P, P], F16, tag="wrm")
            xq_t = [
                consts.tile([P, DT, FREE], F16, tag=f"xq{c}", name=f"xq{c}")
                for c in range(NCH)
            ]
            xc_t = [
                consts.tile([P, DT, FREE], F16, tag=f"xc{c}", name=f"xc{c}")
                for c in range(NCH)
            ]

            # ---------------- input DMA burst: everything issues up front,
            # split across the two hardware DGE queues.  Neither queue has a
            # waiting DMA in front of an input load.  Most-urgent first.
            # sync queue: kv path (xc chunks feed the B-matmul deadline
            # chain), then the non-urgent q-path chunks.
            H = DT // 2
            nc.sync.dma_start(wkv_sb, wkv[:])
            # chunk 0 lands in halves so the projections start ~3us earlier
            nc.sync.dma_start(xc_t[0][:, :H, :], xc[:, 0, :H, :])
            nc.sync.dma_start(xc_t[0][:, H:, :], xc[:, 0, H:, :])
            for c in range(1, NCH):
                nc.sync.dma_start(xc_t[c], xc[:, c, :, :])
            for c in range(1, NCH):
                nc.sync.dma_start(xq_t[c], xq[:, c, :, :])
            nc.sync.dma_start(wo_sb, wo[:])
            # scalar/Act queue (idle until the first exp): only what the
            # first B matmul needs, so the issue cost never delays exp(0).
            nc.scalar.dma_start(wq_sb, wq[:])
            nc.scalar.dma_start(xq_t[0][:, :H, :], xq[:, 0, :H, :])
            nc.scalar.dma_start(xq_t[0][:, H:, :], xq[:, 0, H:, :])
            nc.scalar.dma_start(ident, cid[:])

            # vector-side const init + HAM warm-up (no DMA dependencies:
            # wrm memset feeds dummy matmuls that spin the PE clock up while
            # the inputs stream; a tiny exp pulls the ACT table load early).
            nc.vector.memset(wrm, 0.125)
            nc.vector.memset(vp, 1.0)
            nc.vector.memset(vp2, 1.0)
            dum = consts.tile([P, 8], F16, tag="dum")
            nc.scalar.activation(
                dum, wrm[:, :8], mybir.ActivationFunctionType.Exp, bias=0.0, scale=1.0
            )
            warm = psA.tile([P, P], F32, tag="pa", name="warm")
            for i in range(20):
                nc.tensor.matmul(warm, wrm, wrm, start=(i == 0), stop=(i == 19))

            # ---------------- fill-work machinery (PE slack consumers)
            fills = deque()  # anytime work: projections, transposes, dma issues
            late = deque()  # dep-settled-late work: norm chains, E pieces

            def pop_fill(n=1):
                for _ in range(n):
                    if not fills:
                        return
                    fills.popleft()()

            # D matmuls for one pipelined unit (two heads, K=128, N=512)
            def emit_d(pd0, pd1, pt, t):
                nc.tensor.matmul(
                    pd0, vp[:, t, :], pt[:, :BLK],
                    start=(t == 0), stop=(t == NTK - 1), skip_group_check=True,
                )
                nc.tensor.matmul(
                    pd1, vp2[:, t, :], pt[:, BLK:],
                    start=(t == 0), stop=(t == NTK - 1), skip_group_check=True,
                )

            # kv projection chunk: 8 K-tiles -> kv[:, cs]; k2 shift; transposes
            def kv_chunk_pieces(c):
                cs = slice(c * FREE, (c + 1) * FREE)
                st = {}

                def pk(i0):
                    def p():
                        if i0 == 0:
                            st["pkv"] = psA.tile([P, FREE], F32, tag="pa", name="pkv")
                        for i in range(i0, i0 + 2):
                            nc.tensor.matmul(
                                st["pkv"], wkv_sb[:, i, :], xc_t[c][:, i, :],
                                start=(i == 0), stop=(i == DT - 1),
                            )
                        if i0 == DT - 2:
                            nc.vector.tensor_copy(kv[:, cs], st["pkv"])

                    return p

                def p4():
                    # kT -> partitions 64-127 of k2 via the block-swap matmul
                    psK = psA.tile([P, FREE], F32, tag="pa", name="psK")
                    nc.tensor.matmul(psK, ident[:HEAD_DIM, :], kv[:HEAD_DIM, cs])
                    nc.vector.tensor_copy(k2[HEAD_DIM:, cs], psK[HEAD_DIM:, :])

                def p3():
                    # transpose the 4 v tiles of this chunk, batch-copy to vp/vp2
                    pvb = psA.tile([P, 4 * HEAD_DIM], F16, tag="pa", name="pvb")
                    for k in range(4):
                        ts_ = slice((4 * c + k) * P, (4 * c + k + 1) * P)
                        nc.tensor.transpose(
                            pvb[:, k * HEAD_DIM : (k + 1) * HEAD_DIM],
                            kv[HEAD_DIM:, ts_],
                            ident[HEAD_DIM:, :HEAD_DIM],
                        )
                    src = pvb.rearrange("p (k d) -> p k d", k=4)
                    nc.vector.tensor_copy(vp[:, 4 * c : 4 * c + 4, :HEAD_DIM], src)
                    nc.vector.tensor_copy(vp2[:, 4 * c : 4 * c + 4, HEAD_DIM:], src)

                return [pk(0), pk(2), pk(4), pk(6), p4, p3]

            # q projection chunk (one e): 8 K-tiles -> qt[:, e, cs]
            def q_chunk_pieces(c, e):
                cs = slice(c * FREE, (c + 1) * FREE)
                st = {}

                def pq(i0):
                    def p():
                        if i0 == 0:
                            st["pq"] = psA.tile([P, FREE], F32, tag="pa", name="pq")
                        for i in range(i0, i0 + 2):
                            nc.tensor.matmul(
                                st["pq"], wq_sb[:, i, e * P : (e + 1) * P],
                                xq_t[c][:, i, :],
                                start=(i == 0), stop=(i == DT - 1),
                            )
                        if i0 == DT - 2:
                            nc.vector.tensor_copy(qt[:, e, cs], st["pq"])

                    return p

                return [pq(0), pq(2), pq(4), pq(6)]

            # output projection for one tq block: 8 m-tile pieces staged into
            # ySB, then half-block DMA issues.  In tail mode the py PSUM
            # accumulators rotate over 4 banks (psA pair + the pd banks,
            # which are free once the final spill ran) so the E matmuls
            # never stall on the yo casts, and stores are finer-grained so
            # the last transfer is small.
            def e_pieces(blk, tail=False):
                bs = slice(blk * BLK, (blk + 1) * BLK)
                ysb = ypool.tile([P, MT, FREE], F16, tag="ysb", name=f"ysb{blk}")

                def mk(m):
                    def p():
                        if tail and m % 2 == 1:
                            tag = "pd0" if m % 4 == 1 else "pd1"
                            py = psD.tile([P, FREE], F32, tag=tag, name="py")
                        else:
                            py = psA.tile([P, FREE], F32, tag="pa", name="py")
                        for ee in range(ET):
                            nc.tensor.matmul(
                                py, wo_sb[:, ee, m * P : (m + 1) * P], outs[:, ee, bs],
                                start=(ee == 0), stop=(ee == ET - 1),
                            )
                        nc.vector.tensor_copy(ysb[:, m, :], py)

                    return p

                def store(h, nst):
                    def p():
                        ms = slice(h * (MT // nst), (h + 1) * (MT // nst))
                        nc.sync.dma_start(yt[:, blk, ms, :], ysb[:, ms, :])

                    return p

                pieces = [mk(m) for m in range(MT)]
                if tail:
                    out = []
                    for m in range(MT):
                        out.append(pieces[m])
                        if m % 2 == 1:
                            out.append(store(m // 2, 4))
                    return out
                return pieces + [store(0, 2), store(1, 2)]

            # spill pd0/pd1 for one section into aligned full-partition
            # tiles: rawN = [AV_even | AV_odd], rawD = [den_odd | den_even]
            # (reciprocal_approx_fast silently corrupts partition-offset
            # operands, so the custom op must see full offset-0 tiles).
            def spill(dp0, dp1):
                rawN = recpool.tile([P, BLK], F32, tag="rawN", name="rawN")
                rawD = recpool.tile([P, BLK], F32, tag="rawD", name="rawD")
                nc.vector.tensor_copy(rawN[:HEAD_DIM, :], dp0[:HEAD_DIM, :])
                nc.vector.tensor_copy(rawN[HEAD_DIM:, :], dp1[HEAD_DIM:, :])
                nc.vector.tensor_copy(rawD[:HEAD_DIM, :], dp1[:HEAD_DIM, :])
                nc.vector.tensor_copy(rawD[HEAD_DIM:, :], dp0[HEAD_DIM:, :])
                return rawN, rawD

            # normalize chain for one section (deferred into the late window
            # of the following section): one full-partition fast recip, two
            # half-swap broadcast DMAs, one full-width multiply.
            def norm_chain(sec, rawN, rawD):
                blk, e = divmod(sec, ET)
                bs = slice(blk * BLK, (blk + 1) * BLK)

                def p():
                    recD = recpool.tile([P, BLK], F32, tag="recD", name="recD")
                    recS = recpool.tile([P, BLK], F32, tag="recS", name="recS")
                    nc.vector.reciprocal_approx_fast(recD, rawD)
                    nc.sync.dma_start(recS[:HEAD_DIM, :], recD[HEAD_DIM:, :])
                    nc.sync.dma_start(recS[HEAD_DIM:, :], recD[:HEAD_DIM, :])
                    nc.vector.tensor_mul(outs[:, e, bs], rawN, recS)

                return p

            # ---------------- lead-in PE work: kv chunk 0 + q chunk 0 (e=0)
            # inline; everything else is fills with deadline-ordered layout.
            # interleaved so the PE consumes each half-chunk DMA as it lands
            kc0 = kv_chunk_pieces(0)
            q00 = q_chunk_pieces(0, 0)
            kc0[0]()
            kc0[1]()
            q00[0]()
            q00[1]()
            kc0[2]()
            kc0[3]()
            q00[2]()
            q00[3]()
            kc0[4]()  # k2 shift for chunk 0 (B(0) reads it)

            # Fill deadline order for section 0 (2 pops/unit):
            #   kc0.p3 (v transposes, before D(0) at u=2) -> t=0
            #   kc{c}: pk x4 + p4 (k2 shift) before B(4c); p3 before D(4c).
            fills.append(kc0[5])
            fills.extend(kv_chunk_pieces(1))
            fills.extend(kv_chunk_pieces(2))
            fills.extend(kv_chunk_pieces(3))
            fills.extend(q_chunk_pieces(0, 1))
            for e in range(ET):
                fills.extend(q_chunk_pieces(1, e))
            for e in range(ET):
                fills.extend(q_chunk_pieces(2, e))
            for e in range(ET):
                fills.extend(q_chunk_pieces(3, e))

            # ---------------- the continuous BCD pipeline over 128 units
            units = [(sec, t) for sec in range(NSEC) for t in range(NTK)]
            pending = deque()  # (pd0, pd1, pt, t, sec)
            pd_cur = None
            for u, (sec, t) in enumerate(units):
                blk, e = divmod(sec, ET)
                bs = slice(blk * BLK, (blk + 1) * BLK)
                if t == 0:
                    pd_cur = (
                        psD.tile([P, BLK], F32, tag="pd0", name="pd0"),
                        psD.tile([P, BLK], F32, tag="pd1", name="pd1"),
                    )
                pb = psS.tile([P, 2 * BLK], F32, tag="pb", name="pb")
                # B: two K=64 row-group matmuls, concurrent in the array
                nc.tensor.matmul(pb[:, :BLK], kv[:HEAD_DIM, t * P : (t + 1) * P],
                                 qt[:HEAD_DIM, e, bs])
                nc.tensor.matmul(pb[:, BLK:], k2[HEAD_DIM:, t * P : (t + 1) * P],
                                 qt[HEAD_DIM:, e, bs])
                if len(pending) >= 2:
                    dp0, dp1, dpt, dt_, dsec = pending.popleft()
                    emit_d(dp0, dp1, dpt, dt_)
                    if dt_ == NTK - 1 and dsec < NSEC - 1:
                        # section dsec fully accumulated: spill pd -> raw
                        # (frees the PSUM banks for this section's own Ds),
                        # queue the normalize + block-complete E work.
                        rawN, rawD = spill(dp0, dp1)
                        late.append(norm_chain(dsec, rawN, rawD))
                        if dsec % ET == ET - 1 and dsec >= 1:
                            late.extend(e_pieces(dsec // ET))
                pt = ptpool.tile([P, 2 * BLK], F16, tag="pt", name="pt")
                nc.scalar.activation(
                    pt, pb, mybir.ActivationFunctionType.Exp, bias=0.0, scale=SCALE
                )
                pending.append((pd_cur[0], pd_cur[1], pt, t, sec))
                if sec == 0:
                    pop_fill(2)
                elif 5 <= t <= 14 and late:
                    late.popleft()()
                elif t >= 2:
                    pop_fill(1)

            # ---------------- tail: drain last two Ds, normalize the final
            # section straight out of PSUM (fast recip), output-project the
            # last block, store.
            final_pd = None
            while pending:
                dp0, dp1, dpt, dt_, dsec = pending.popleft()
                emit_d(dp0, dp1, dpt, dt_)
                final_pd = (dp0, dp1)
            while late:
                late.popleft()()
            # tail normalize, minimum latency: den spill only, fast recip,
            # fp16 cast + PE block-swap (no DMA round trip), muls straight
            # from the pd PSUM banks.
            dp0, dp1 = final_pd
            e, bs = 1, slice((NBLK - 1) * BLK, NBLK * BLK)
            rawD = recpool.tile([P, BLK], F32, tag="rawD", name="rawD")
            nc.vector.tensor_copy(rawD[:HEAD_DIM, :], dp1[:HEAD_DIM, :])
            nc.vector.tensor_copy(rawD[HEAD_DIM:, :], dp0[HEAD_DIM:, :])
            recD = recpool.tile([P, BLK], F32, tag="recD", name="recD")
            nc.vector.reciprocal_approx_fast(recD, rawD)
            recH = recpool.tile([P, BLK], F16, tag="recS", name="recH")
            nc.vector.tensor_copy(recH, recD)
            psR = psA.tile([P, BLK], F32, tag="pa", name="psR")
            nc.tensor.matmul(psR, ident, recH)
            recS = recpool.tile([P, BLK], F32, tag="rawN", name="recSf")
            nc.vector.tensor_copy(recS, psR)
            nc.vector.tensor_mul(outs[:HEAD_DIM, e, bs], dp0[:HEAD_DIM, :],
                                 recS[:HEAD_DIM, :])
            nc.vector.tensor_mul(outs[HEAD_DIM:, e, bs], dp1[HEAD_DIM:, :],
                                 recS[HEAD_DIM:, :])
            for piece in e_pieces(NBLK - 1, tail=True):
                piece()
            while fills:
                pop_fill()

    nc.finalize()
    return nc


_NC_CACHE = None


def _get_nc():
    global _NC_CACHE
    if _NC_CACHE is None:
        _NC_CACHE = build_bass()
    return _NC_CACHE


def _cid2():
    z = np.zeros((HEAD_DIM, HEAD_DIM), dtype=np.float16)
    i = np.eye(HEAD_DIM, dtype=np.float16)
    return np.block([[z, i], [i, z]])


def _chunked(xT):
    """[D_MODEL, T] -> [P, NCH, DT, FREE] with row i*P+p at [p, :, i, :]:
    each partition's chunk data contiguous for large-descriptor DMA."""
    return np.ascontiguousarray(
        xT.reshape(DT, P, NCH, FREE).transpose(1, 2, 0, 3)
    ).astype(np.float16)


def _wtiles(wT):
    """[D_MODEL, E] -> [P, DT, E]"""
    return np.ascontiguousarray(
        wT.reshape(DT, P, wT.shape[1]).transpose(1, 0, 2)
    ).astype(np.float16)


def shard_inputs(query, context, Wq, Wk, Wv, Wo):
    """host-side sharding: 8 cores = batch(2) x kv-group(4)"""
    in_maps = []
    xqh = [_chunked(np.asarray(query[b]).T) for b in range(B)]
    xch = [_chunked(np.asarray(context[b]).T) for b in range(B)]
    for core in range(N_CORES):
        b, g = divmod(core, GROUPS)
        wqh = _wtiles(Wq[g * DQ : (g + 1) * DQ, :].T)
        wkvh = _wtiles(
            np.concatenate(
                [
                    Wk[g * HEAD_DIM : (g + 1) * HEAD_DIM, :],
                    Wv[g * HEAD_DIM : (g + 1) * HEAD_DIM, :],
                ],
                axis=0,
            ).T
        )
        woT = Wo[:, g * DQ : (g + 1) * DQ].T  # [DQ, D_MODEL]
        woh = np.ascontiguousarray(
            woT.reshape(ET, P, D_MODEL).transpose(1, 0, 2)
        ).astype(np.float16)
        in_maps.append(
            {
                "xqh": xqh[b],
                "xch": xch[b],
                "wqh": wqh,
                "wkvh": wkvh,
                "woh": woh,
                "cid2": _cid2(),
            }
        )
    return in_maps


def kernel(query, context, Wq, Wk, Wv, Wo, _want_profile=False):
    from concourse.bass_utils import run_bass_kernel_spmd

    nc = _get_nc()
    in_maps = shard_inputs(query, context, Wq, Wk, Wv, Wo)
    res = run_bass_kernel_spmd(
        nc, in_maps, core_ids=list(range(N_CORES)), trace=_want_profile
    )
    out = np.zeros((B, TQ, D_MODEL), dtype=np.float32)
    for core in range(N_CORES):
        b = core // GROUPS
        yh = res.results[core]["yh"].astype(np.float32)
        yT = yh.transpose(2, 0, 1, 3).reshape(D_MODEL, TQ)
        out[b] += yT.T
    if _want_profile:
        return out, res
    return out


# revision 27
# speedup vs baseline: 1.1955x; 1.0048x over previous
"""Cross-attention (GQA) Trainium2 Bass kernel — pipelined v3.

Problem: B=2, Tq=Tkv=2048, D_MODEL=1024, 16 query heads / 4 kv heads,
head_dim=64.  Sharded over 8 NeuronCores as batch(2) x kv-group(4); each
core computes 4 query heads + its single kv head and a partial output
projection (Wo row-split by head group); partials are summed on host.

Dataflow (feature dim on SBUF partitions end-to-end, no big transposes):

  A: qT[e,t] = WqT.T @ xqT,  kvT = WkvT.T @ xcT   (weights stationary)
     v[tk,dv] via PE-transpose of vT tiles; vp=[v|1], vp2=[1|v]
  B: per (sec=blk,e) unit t: pb[128,1024] = two K=64 row-group matmuls
     (h_even rows 0-63 -> cols 0:512, h_odd rows 64-127 -> cols
     512:1024), concurrent in the PE array.
  C: pt = exp(pb/8) one ScalarE instruction per unit (FD=1024).
  D: pd_h[128,512] += vp_t.T @ pt_half; ones-columns give the softmax
     denominators in the complementary 64 partitions.
  E: yT += WoT.T @ (pd*recip(den)), row-split by head pair.

v3 over v2:
  - ONE continuous 128-unit software pipeline (B(u), D(u-2), exp(u))
    across all 8 sections: no PE drain / exp restart bubble at section
    boundaries.
  - reciprocal_approx_fast (custom DVE op, ~5x faster than RECIPROCAL)
    in the softmax-denominator normalize chains.
  - Input DMAs split across BOTH hardware DGE queues: xq/wq/wo/ident
    issue from the (idle during lead-in) scalar/Act queue, xc/wkv from
    the sync queue, all unconditionally at kernel start so no waiting
    DMA ever sits in front of an input load.
  - Norm chains and output-projection (E) pieces pop from a dedicated
    "late" queue only at units t in [5..14] of the FOLLOWING section,
    when their DVE-side producers have long retired - they never stall
    the in-order PE.
  - y stores staged per tq-block in SBUF and written with 2 half-block
    DMAs (8 DMA issues/block -> 2), keeping the sync queue uncongested
    so the rec-broadcast DMAs on the norm critical path fire promptly.

ScalarE (the 1 elem/cycle/lane exp bottleneck, ~143us) paces the
kernel; the PE stays dense and HAM-warm.
"""

import os
import sys
from collections import deque

import numpy as np

for _p in ("/opt/trn_rl_repo",):
    if _p not in sys.path and os.path.isdir(_p):
        sys.path.insert(0, _p)

import concourse.bass as bass
import concourse.bacc as bacc
import concourse.mybir as mybir
from concourse.tile import TileContext

# ---------------------------------------------------------------- problem dims
B = 2
TQ = 2048
TKV = 2048
D_MODEL = 1024
N_HEADS = 16
N_KV_HEADS = 4
HEAD_DIM = 64
N_CORES = 8
GROUPS = N_KV_HEADS  # kv groups = 4
HEADS_PER_DEV = N_HEADS // GROUPS  # 4
DQ = HEADS_PER_DEV * HEAD_DIM  # 256
DKV = 2 * HEAD_DIM  # 128 (k rows + v rows stacked)
SCALE = 1.0 / float(np.sqrt(HEAD_DIM))

P = 128
FREE = 512  # matmul moving-operand chunk / tq block width
BLK = 512
NBLK = TQ // BLK  # 4 tq blocks
DT = D_MODEL // P  # 8 d-tiles
ET = DQ // P  # 2 e-tiles (query head pairs)
NCH = TQ // FREE  # 4 x chunks of 512
NTK = TKV // P  # 16 tk tiles
MT = D_MODEL // P  # 8 output m-tiles
NSEC = NBLK * ET  # 8 sections
NU = NSEC * NTK  # 128 pipelined units

F32 = mybir.dt.float32
F16 = mybir.dt.float16


def build_bass():
    nc = bacc.Bacc()

    # all inputs/outputs are host-pre-arranged to be contiguous per SBUF
    # partition: each load/store is ~128 large descriptors, not 1024 small
    # ones (DGE issue cost and HBM efficiency both scale with that).
    xq = nc.declare_dram_parameter("xqh", [P, NCH, DT, FREE], F16, isOutput=False)
    xc = nc.declare_dram_parameter("xch", [P, NCH, DT, FREE], F16, isOutput=False)
    wq = nc.declare_dram_parameter("wqh", [P, DT, DQ], F16, isOutput=False)
    wkv = nc.declare_dram_parameter("wkvh", [P, DT, DKV], F16, isOutput=False)
    wo = nc.declare_dram_parameter("woh", [P, ET, D_MODEL], F16, isOutput=False)
    # block-swap matrix [[0,I64],[I64,0]]: cid2[64:, :64] is a plain I64 for
    # PE transposes; the full matrix PE-shifts kT from partitions 0-63 into
    # 64-127 (replacing a scheduler-hostile SBUF->SBUF broadcast DMA).
    cid = nc.declare_dram_parameter("cid2", [P, P], F16, isOutput=False)
    yt = nc.declare_dram_parameter("yh", [P, NBLK, MT, FREE], F16, isOutput=True)

    with TileContext(nc) as tc:
        with (
            tc.tile_pool(name="consts", bufs=1) as consts,
            tc.tile_pool(name="pt", bufs=4) as ptpool,
            tc.tile_pool(name="rec", bufs=2) as recpool,
            tc.tile_pool(name="yout", bufs=2) as ypool,
            tc.tile_pool(name="psS", bufs=2, space="PSUM") as psS,
            tc.tile_pool(name="psD", bufs=1, space="PSUM") as psD,
            tc.tile_pool(name="psA", bufs=2, space="PSUM") as psA,
        ):
            # ---------------- persistent tiles
            qt = consts.tile([P, ET, TQ], F16, tag="qt")  # head pair per e
            kv = consts.tile([P, TKV], F16, tag="kv")  # rows 0-63 kT, 64-127 vT
            k2 = consts.tile([P, TKV], F16, tag="k2")  # rows 64-127 = kT copy
            vp = consts.tile([P, NTK, P], F16, tag="vp")  # [v | ones]
            vp2 = consts.tile([P, NTK, P], F16, tag="vp2")  # [ones | v]
            outs = consts.tile([P, ET, TQ], F16, tag="outs")  # normalized outT
            ident = consts.tile([P, P], F16, tag="ident")
            wkv_sb = consts.tile([P, DT, DKV], F16, tag="wkv")
            wq_sb = consts.tile([P, DT, DQ], F16, tag="wq")
            wo_sb = consts.tile([P, ET, D_MODEL], F16, tag="wo")
            wrm = consts.tile([P, P], F16, tag="wrm")
            xq_t = [
                consts.tile([P, DT, FREE], F16, tag=f"xq{c}", name=f"xq{c}")
                for c in range(NCH)
            ]
            xc_t = [
                consts.tile([P, DT, FREE], F16, tag=f"xc{c}", name=f"xc{c}")
                for c in range(NCH)
            ]

            # ---------------- input DMA burst: everything issues up front,
            # split across the two hardware DGE queues.  Neither queue has a
            # waiting DMA in front of an input load.  Most-urgent first.
            # sync queue: kv path (xc chunks feed the B-matmul deadline
            # chain), then the non-urgent q-path chunks.
            # chunk 0 lands in quarters (one per projection piece) so the PE
            # ramps continuously instead of stalling on whole-chunk arrivals
            nc.sync.dma_start(wkv_sb, wkv[:])
            for qi in range(4):
                nc.sync.dma_start(
                    xc_t[0][:, 2 * qi : 2 * qi + 2, :], xc[:, 0, 2 * qi : 2 * qi + 2, :]
                )
            H = DT // 2
            nc.sync.dma_start(xc_t[1][:, :H, :], xc[:, 1, :H, :])
            nc.sync.dma_start(xc_t[1][:, H:, :], xc[:, 1, H:, :])
            for c in range(2, NCH):
                nc.sync.dma_start(xc_t[c], xc[:, c, :, :])
            for c in range(1, NCH):
                nc.sync.dma_start(xq_t[c], xq[:, c, :, :])
            nc.sync.dma_start(wo_sb, wo[:])
            # scalar/Act queue (idle until the first exp): only what the
            # first B matmul needs, so the issue cost never delays exp(0).
            # ident first: it gates the k2 shift and the v transposes.
            nc.scalar.dma_start(ident, cid[:])
            nc.scalar.dma_start(wq_sb, wq[:])
            for qi in range(4):
                nc.scalar.dma_start(
                    xq_t[0][:, 2 * qi : 2 * qi + 2, :], xq[:, 0, 2 * qi : 2 * qi + 2, :]
                )

            # vector-side const init + HAM warm-up (no DMA dependencies:
            # wrm memset feeds dummy matmuls that spin the PE clock up while
            # the inputs stream; a tiny exp pulls the ACT table load early).
            nc.vector.memset(wrm, 0.125)
            nc.vector.memset(vp, 1.0)
            nc.vector.memset(vp2, 1.0)
            dum = consts.tile([P, 8], F16, tag="dum")
            nc.scalar.activation(
                dum, wrm[:, :8], mybir.ActivationFunctionType.Exp, bias=0.0, scale=1.0
            )
            warm = psA.tile([P, P], F32, tag="pa", name="warm")
            for i in range(20):
                nc.tensor.matmul(warm, wrm, wrm, start=(i == 0), stop=(i == 19))

            # ---------------- fill-work machinery (PE slack consumers)
            fills = deque()  # anytime work: projections, transposes, dma issues
            late = deque()  # dep-settled-late work: norm chains, E pieces

            def pop_fill(n=1):
                for _ in range(n):
                    if not fills:
                        return
                    fills.popleft()()

            # D matmuls for one pipelined unit (two heads, K=128, N=512)
            def emit_d(pd0, pd1, pt, t):
                nc.tensor.matmul(
                    pd0, vp[:, t, :], pt[:, :BLK],
                    start=(t == 0), stop=(t == NTK - 1), skip_group_check=True,
                )
                nc.tensor.matmul(
                    pd1, vp2[:, t, :], pt[:, BLK:],
                    start=(t == 0), stop=(t == NTK - 1), skip_group_check=True,
                )

            # kv projection chunk: 8 K-tiles -> kv[:, cs]; k2 shift; transposes
            def kv_chunk_pieces(c):
                cs = slice(c * FREE, (c + 1) * FREE)
                st = {}

                def pk(i0):
                    def p():
                        if i0 == 0:
                            st["pkv"] = psA.tile([P, FREE], F32, tag="pa", name="pkv")
                        for i in range(i0, i0 + 2):
                            nc.tensor.matmul(
                                st["pkv"], wkv_sb[:, i, :], xc_t[c][:, i, :],
                                start=(i == 0), stop=(i == DT - 1),
                            )
                        if i0 == DT - 2:
                            nc.vector.tensor_copy(kv[:, cs], st["pkv"])

                    return p

                def p4():
                    # kT -> partitions 64-127 of k2 via the block-swap matmul
                    psK = psA.tile([P, FREE], F32, tag="pa", name="psK")
                    nc.tensor.matmul(psK, ident[:HEAD_DIM, :], kv[:HEAD_DIM, cs])
                    nc.vector.tensor_copy(k2[HEAD_DIM:, cs], psK[HEAD_DIM:, :])

                def p3():
                    # transpose the 4 v tiles of this chunk, batch-copy to vp/vp2
                    pvb = psA.tile([P, 4 * HEAD_DIM], F16, tag="pa", name="pvb")
                    for k in range(4):
                        ts_ = slice((4 * c + k) * P, (4 * c + k + 1) * P)
                        nc.tensor.transpose(
                            pvb[:, k * HEAD_DIM : (k + 1) * HEAD_DIM],
                            kv[HEAD_DIM:, ts_],
                            ident[HEAD_DIM:, :HEAD_DIM],
                        )
                    src = pvb.rearrange("p (k d) -> p k d", k=4)
                    nc.vector.tensor_copy(vp[:, 4 * c : 4 * c + 4, :HEAD_DIM], src)
                    nc.vector.tensor_copy(vp2[:, 4 * c : 4 * c + 4, HEAD_DIM:], src)

                return [pk(0), pk(2), pk(4), pk(6), p4, p3]

            # q projection chunk (one e): 8 K-tiles -> qt[:, e, cs]
            def q_chunk_pieces(c, e):
                cs = slice(c * FREE, (c + 1) * FREE)
                st = {}

                def pq(i0):
                    def p():
                        if i0 == 0:
                            st["pq"] = psA.tile([P, FREE], F32, tag="pa", name="pq")
                        for i in range(i0, i0 + 2):
                            nc.tensor.matmul(
                                st["pq"], wq_sb[:, i, e * P : (e + 1) * P],
                                xq_t[c][:, i, :],
                                start=(i == 0), stop=(i == DT - 1),
                            )
                        if i0 == DT - 2:
                            nc.vector.tensor_copy(qt[:, e, cs], st["pq"])

                    return p

                return [pq(0), pq(2), pq(4), pq(6)]

            # output projection for one tq block: 8 m-tile pieces staged into
            # ySB, then half-block DMA issues.  In tail mode the py PSUM
            # accumulators rotate over 4 banks (psA pair + the pd banks,
            # which are free once the final spill ran) so the E matmuls
            # never stall on the yo casts, and stores are finer-grained so
            # the last transfer is small.
            def e_pieces(blk, tail=False):
                bs = slice(blk * BLK, (blk + 1) * BLK)
                ysb = ypool.tile([P, MT, FREE], F16, tag="ysb", name=f"ysb{blk}")

                def mk(m):
                    def p():
                        if tail and m % 2 == 1:
                            tag = "pd0" if m % 4 == 1 else "pd1"
                            py = psD.tile([P, FREE], F32, tag=tag, name="py")
                        else:
                            py = psA.tile([P, FREE], F32, tag="pa", name="py")
                        for ee in range(ET):
                            nc.tensor.matmul(
                                py, wo_sb[:, ee, m * P : (m + 1) * P], outs[:, ee, bs],
                                start=(ee == 0), stop=(ee == ET - 1),
                            )
                        nc.vector.tensor_copy(ysb[:, m, :], py)

                    return p

                def store(h, nst):
                    def p():
                        ms = slice(h * (MT // nst), (h + 1) * (MT // nst))
                        nc.sync.dma_start(yt[:, blk, ms, :], ysb[:, ms, :])

                    return p

                pieces = [mk(m) for m in range(MT)]
                if tail:
                    out = []
                    for m in range(MT):
                        out.append(pieces[m])
                        if m % 2 == 1:
                            out.append(store(m // 2, 4))
                    return out
                return pieces + [store(0, 2), store(1, 2)]

            # spill pd0/pd1 for one section into aligned full-partition
            # tiles: rawN = [AV_even | AV_odd], rawD = [den_odd | den_even]
            # (reciprocal_approx_fast silently corrupts partition-offset
            # operands, so the custom op must see full offset-0 tiles).
            def spill(dp0, dp1):
                rawN = recpool.tile([P, BLK], F32, tag="rawN", name="rawN")
                rawD = recpool.tile([P, BLK], F32, tag="rawD", name="rawD")
                nc.vector.tensor_copy(rawN[:HEAD_DIM, :], dp0[:HEAD_DIM, :])
                nc.vector.tensor_copy(rawN[HEAD_DIM:, :], dp1[HEAD_DIM:, :])
                nc.vector.tensor_copy(rawD[:HEAD_DIM, :], dp1[:HEAD_DIM, :])
                nc.vector.tensor_copy(rawD[HEAD_DIM:, :], dp0[HEAD_DIM:, :])
                return rawN, rawD

            # normalize chain for one section (deferred into the late window
            # of the following section): one full-partition fast recip, two
            # half-swap broadcast DMAs, one full-width multiply.
            def norm_chain(sec, rawN, rawD):
                blk, e = divmod(sec, ET)
                bs = slice(blk * BLK, (blk + 1) * BLK)

                def p():
                    recD = recpool.tile([P, BLK], F32, tag="recD", name="recD")
                    recS = recpool.tile([P, BLK], F32, tag="recS", name="recS")
                    nc.vector.reciprocal_approx_fast(recD, rawD)
                    nc.sync.dma_start(recS[:HEAD_DIM, :], recD[HEAD_DIM:, :])
                    nc.sync.dma_start(recS[HEAD_DIM:, :], recD[:HEAD_DIM, :])
                    nc.vector.tensor_mul(outs[:, e, bs], rawN, recS)

                return p

            # ---------------- lead-in PE work: kv chunk 0 + q chunk 0 (e=0)
            # inline; everything else is fills with deadline-ordered layout.
            # interleaved so the PE consumes each quarter-chunk DMA as it lands
            kc0 = kv_chunk_pieces(0)
            q00 = q_chunk_pieces(0, 0)
            kc0[0]()
            q00[0]()
            kc0[1]()
            q00[1]()
            kc0[2]()
            q00[2]()
            kc0[3]()
            q00[3]()
            kc0[4]()  # k2 shift for chunk 0 (B(0) reads it)

            # Fill deadline order for section 0 (2 pops/unit):
            #   kc0.p3 (v transposes, before D(0) at u=2) -> t=0
            #   kc{c}: pk x4 + p4 (k2 shift) before B(4c); p3 before D(4c).
            fills.append(kc0[5])
            fills.extend(kv_chunk_pieces(1))
            fills.extend(kv_chunk_pieces(2))
            fills.extend(kv_chunk_pieces(3))
            fills.extend(q_chunk_pieces(0, 1))
            for e in range(ET):
                fills.extend(q_chunk_pieces(1, e))
            for e in range(ET):
                fills.extend(q_chunk_pieces(2, e))
            for e in range(ET):
                fills.extend(q_chunk_pieces(3, e))

            # ---------------- the continuous BCD pipeline over 128 units
            units = [(sec, t) for sec in range(NSEC) for t in range(NTK)]
            pending = deque()  # (pd0, pd1, pt, t, sec)
            pd_cur = None
            for u, (sec, t) in enumerate(units):
                blk, e = divmod(sec, ET)
                bs = slice(blk * BLK, (blk + 1) * BLK)
                if t == 0:
                    pd_cur = (
                        psD.tile([P, BLK], F32, tag="pd0", name="pd0"),
                        psD.tile([P, BLK], F32, tag="pd1", name="pd1"),
                    )
                pb = psS.tile([P, 2 * BLK], F32, tag="pb", name="pb")
                # B: two K=64 row-group matmuls, concurrent in the array
                nc.tensor.matmul(pb[:, :BLK], kv[:HEAD_DIM, t * P : (t + 1) * P],
                                 qt[:HEAD_DIM, e, bs])
                nc.tensor.matmul(pb[:, BLK:], k2[HEAD_DIM:, t * P : (t + 1) * P],
                                 qt[HEAD_DIM:, e, bs])
                if len(pending) >= 2:
                    dp0, dp1, dpt, dt_, dsec = pending.popleft()
                    emit_d(dp0, dp1, dpt, dt_)
                    if dt_ == NTK - 1 and dsec < NSEC - 1:
                        # section dsec fully accumulated: spill pd -> raw
                        # (frees the PSUM banks for this section's own Ds),
                        # queue the normalize + block-complete E work.
                        rawN, rawD = spill(dp0, dp1)
                        late.append(norm_chain(dsec, rawN, rawD))
                        if dsec % ET == ET - 1 and dsec >= 1:
                            late.extend(e_pieces(dsec // ET))
                pt = ptpool.tile([P, 2 * BLK], F16, tag="pt", name="pt")
                nc.scalar.activation(
                    pt, pb, mybir.ActivationFunctionType.Exp, bias=0.0, scale=SCALE
                )
                pending.append((pd_cur[0], pd_cur[1], pt, t, sec))
                if sec == 0:
                    # 2/unit covers the kv deadline chain + q(0,1); beyond
                    # that, spread the q projections over later sections so
                    # the half-clocked lead-in PE isn't oversubscribed.
                    pop_fill(2 if t < 12 else 1)
                elif 5 <= t <= 14 and late:
                    late.popleft()()
                elif t >= 2:
                    pop_fill(1)

            # ---------------- tail: drain last two Ds, normalize the final
            # section straight out of PSUM (fast recip), output-project the
            # last block, store.
            final_pd = None
            while pending:
                dp0, dp1, dpt, dt_, dsec = pending.popleft()
                emit_d(dp0, dp1, dpt, dt_)
                final_pd = (dp0, dp1)
            # warm-keepers: run during the ~3us norm chain below so the PE
            # P-state stays at full clock for the output projection.
            wt = psA.tile([P, P], F32, tag="pa", name="wt")
            for i in range(24):
                nc.tensor.matmul(wt, wrm, wrm, start=(i == 0), stop=(i == 23))
            while late:
                late.popleft()()
            # tail normalize, minimum latency: den spill only, fast recip,
            # fp16 cast + PE block-swap (no DMA round trip), muls straight
            # from the pd PSUM banks.
            dp0, dp1 = final_pd
            e, bs = 1, slice((NBLK - 1) * BLK, NBLK * BLK)
            rawD = recpool.tile([P, BLK], F32, tag="rawD", name="rawD")
            nc.vector.tensor_copy(rawD[:HEAD_DIM, :], dp1[:HEAD_DIM, :])
            nc.vector.tensor_copy(rawD[HEAD_DIM:, :], dp0[HEAD_DIM:, :])
            recD = recpool.tile([P, BLK], F32, tag="recD", name="recD")
            nc.vector.reciprocal_approx_fast(recD, rawD)
            recH = recpool.tile([P, BLK], F16, tag="recS", name="recH")
            nc.vector.tensor_copy(recH, recD)
            psR = psA.tile([P, BLK], F32, tag="pa", name="psR")
            nc.tensor.matmul(psR, ident, recH)
            recS = recpool.tile([P, BLK], F32, tag="rawN", name="recSf")
            nc.vector.tensor_copy(recS, psR)
            nc.vector.tensor_mul(outs[:HEAD_DIM, e, bs], dp0[:HEAD_DIM, :],
                                 recS[:HEAD_DIM, :])
            nc.vector.tensor_mul(outs[HEAD_DIM:, e, bs], dp1[HEAD_DIM:, :],
                                 recS[HEAD_DIM:, :])
            for piece in e_pieces(NBLK - 1, tail=True):
                piece()
            while fills:
                pop_fill()

    nc.finalize()
    return nc


_NC_CACHE = None


def _get_nc():
    global _NC_CACHE
    if _NC_CACHE is None:
        _NC_CACHE = build_bass()
    return _NC_CACHE


def _cid2():
    z = np.zeros((HEAD_DIM, HEAD_DIM), dtype=np.float16)
    i = np.eye(HEAD_DIM, dtype=np.float16)
    return np.block([[z, i], [i, z]])


def _chunked(xT):
    """[D_MODEL, T] -> [P, NCH, DT, FREE] with row i*P+p at [p, :, i, :]:
    each partition's chunk data contiguous for large-descriptor DMA."""
    return np.ascontiguousarray(
        xT.reshape(DT, P, NCH, FREE).transpose(1, 2, 0, 3)
    ).astype(np.float16)


def _wtiles(wT):
    """[D_MODEL, E] -> [P, DT, E]"""
    return np.ascontiguousarray(
        wT.reshape(DT, P, wT.shape[1]).transpose(1, 0, 2)
    ).astype(np.float16)


def shard_inputs(query, context, Wq, Wk, Wv, Wo):
    """host-side sharding: 8 cores = batch(2) x kv-group(4)"""
    in_maps = []
    xqh = [_chunked(np.asarray(query[b]).T) for b in range(B)]
    xch = [_chunked(np.asarray(context[b]).T) for b in range(B)]
    for core in range(N_CORES):
        b, g = divmod(core, GROUPS)
        wqh = _wtiles(Wq[g * DQ : (g + 1) * DQ, :].T)
        wkvh = _wtiles(
            np.concatenate(
                [
                    Wk[g * HEAD_DIM : (g + 1) * HEAD_DIM, :],
                    Wv[g * HEAD_DIM : (g + 1) * HEAD_DIM, :],
                ],
                axis=0,
            ).T
        )
        woT = Wo[:, g * DQ : (g + 1) * DQ].T  # [DQ, D_MODEL]
        woh = np.ascontiguousarray(
            woT.reshape(ET, P, D_MODEL).transpose(1, 0, 2)
        ).astype(np.float16)
        in_maps.append(
            {
                "xqh": xqh[b],
                "xch": xch[b],
                "wqh": wqh,
                "wkvh": wkvh,
                "woh": woh,
                "cid2": _cid2(),
            }
        )
    return in_maps


def kernel(query, context, Wq, Wk, Wv, Wo, _want_profile=False):
    from concourse.bass_utils import run_bass_kernel_spmd

    nc = _get_nc()
    in_maps = shard_inputs(query, context, Wq, Wk, Wv, Wo)
    res = run_bass_kernel_spmd(
        nc, in_maps, core_ids=list(range(N_CORES)), trace=_want_profile
    )
    out = np.zeros((B, TQ, D_MODEL), dtype=np.float32)
    for core in range(N_CORES):
        b = core // GROUPS
        yh = res.results[core]["yh"].astype(np.float32)
        yT = yh.transpose(2, 0, 1, 3).reshape(D_MODEL, TQ)
        out[b] += yT.T
    if _want_profile:
        return out, res
    return out
